# revision 2
# baseline (speedup 1.0000x reference)
"""Trainium2 Bass kernel for CustomGNN (fc+relu -> SAGEConv+relu -> SAGEConv -> head).

Single-NEFF SPMD design (8 NeuronCores, one program, per-core data):
  - nodes sharded in 8 contiguous chunks of 12500 (dst-partitioned edges).
  - fc stage sharded: each core computes h0 for its own nodes, then an
    AllGather builds the full [100000, 64] h0 table in every core's HBM.
  - SAGE layer 1: per 512-dst window, edges (sorted by dst, packed into
    128-slot columns) gather h0[src] rows (256B) with dma_gather from 4 src
    chunks (int16 index range); segment-sum via PE matmuls against one-hot
    inv-deg matrices built on DVE (is_equal vs iota over 32-wide sliding seg
    windows); then zT = Wl1 @ aggT + Wr1 @ h0T + bl1, relu -> h1T.
  - SAGE layer 2 + head collapsed to scalars (linearity of the head):
      out[v] = invdeg_v * sum_{u->v} s[u] + t[v]
      s[u] = h1[u] . (head_W @ Wl2),  t[v] = h1[v] . (head_W @ Wr2) + c0
    s is AllGathered (50KB/rank, padded shard stride 12544), then a second
    window pass gathers 64-value blocks of the dense s table (idx = padded
    row >> 6, fits int16 with no chunking), selects the scalar with a DVE
    one-hot over 64, and folds the 64-wide reduction into the PE
    segment-sum matmuls (final ones-vector partition reduce per window).

All index-derived structure (column quotas R, seg-window starts F) is computed
on the host from edge_index and baked into the program; quotas are max-merged
across the 8 cores so the SPMD program is identical on every core.
"""
import math
import time
import numpy as np

from concourse import bacc, mybir
from concourse.tile import TileContext
from concourse.bass_utils import run_bass_kernel_spmd

timings = {}

NSWQ = 2

F32 = mybir.dt.float32
I16 = mybir.dt.int16


class Cfg:
    def __init__(self, n_nodes=100000, in_dim=128, hid=64, ncores=8,
                 win=512, nchunk=4, segw=32, colb=8, xb=4, gb2=48):
        assert n_nodes % ncores == 0
        self.N = n_nodes
        self.D = in_dim
        self.H = hid
        self.NC = ncores
        self.NPC = n_nodes // ncores
        self.WIN = win
        self.NWIN = math.ceil(self.NPC / win)
        self.NCHUNK = nchunk
        self.CHUNK = math.ceil(n_nodes / nchunk)
        assert self.CHUNK <= 32767
        self.SEGW = segw
        self.COLB = colb
        self.XB = xb
        self.GB2 = gb2                        # layer-2 gather block (columns)
        self.NBLK = math.ceil(self.NPC / 128)
        self.SPAD = self.NBLK * 128           # padded shard stride (12544)
        assert self.SPAD % 64 == 0
        self.NB2 = self.SPAD * ncores // 64   # layer-2 table rows ([NB2, 64])


def _schedule_F(R, span, segw):
    if R <= 1:
        return [0] * max(R, 1)
    top = max(span - segw, 0)
    return [min((j * top) // (R - 1), top) for j in range(R)]


def _pack_group(segs, R, F, segw):
    """Greedy-pack sorted segs into R cols of <=128 edges, col j window
    [F[j], F[j]+segw). Returns list of (start, count) per col, or None."""
    n = len(segs)
    ptr = 0
    out = []
    for j in range(R):
        if ptr < n and segs[ptr] < F[j]:
            return None
        hi = np.searchsorted(segs, F[j] + segw, side="left")
        take = min(128, hi - ptr)
        out.append((ptr, take))
        ptr += take
    if ptr != n:
        return None
    return out


def _plan_pass(cfg, dst, seg_all_src, gid_base, nchunk, chunk, tabidx, extra):
    """Generic planning for one gather+segment-sum pass.

    dst: global dst per edge; chunk: chunk id per edge (grouping + gather
    table split); tabidx: per-edge gather index (relative to chunk base
    handled by caller); extra: dict name -> per-edge f32 array baked beside
    (seg, invdeg) into the segv tensor.
    Returns structure dict + per-core idx (int16, wrapped) and segv arrays.
    """
    N, NC, NPC, WIN, NWIN, SEGW = cfg.N, cfg.NC, cfg.NPC, cfg.WIN, cfg.NWIN, cfg.SEGW

    deg = np.bincount(dst, minlength=N)
    inv = (1.0 / np.maximum(deg, 1)).astype(np.float32)

    core = dst // NPC
    local = dst - core * NPC
    win = local // WIN
    seg = (local - win * WIN).astype(np.int32)

    gid = ((core * NWIN + win) * nchunk + chunk).astype(np.int64)
    order = np.lexsort((seg, gid))
    gids = gid[order]
    segs_all = seg[order]
    tab_all = tabidx[order]
    invdst_all = inv[dst[order]].astype(np.float32)
    extra_all = {k: v[order] for k, v in extra.items()}

    ngroups = NC * NWIN * nchunk
    counts = np.bincount(gids, minlength=ngroups)
    starts = np.zeros(ngroups + 1, dtype=np.int64)
    np.cumsum(counts, out=starts[1:])

    def span_of(w):
        return min(WIN, NPC - w * WIN)

    R = [[0] * nchunk for _ in range(NWIN)]
    F = [[None] * nchunk for _ in range(NWIN)]
    packs = {}
    for w in range(NWIN):
        span = span_of(w)
        for c in range(nchunk):
            r = 1
            for k in range(NC):
                g = (k * NWIN + w) * nchunk + c
                r = max(r, math.ceil(max(counts[g], 1) / 128))
            while True:
                f = _schedule_F(r, span, SEGW)
                ok = True
                for k in range(NC):
                    g = (k * NWIN + w) * nchunk + c
                    segs = segs_all[starts[g]:starts[g + 1]]
                    p = _pack_group(segs, r, f, SEGW)
                    if p is None:
                        ok = False
                        break
                    packs[(k, w, c)] = p
                if ok:
                    break
                r += 1
            R[w][c] = r
            F[w][c] = f

    col_off = [[0] * nchunk for _ in range(NWIN)]
    tot = 0
    for w in range(NWIN):
        for c in range(nchunk):
            col_off[w][c] = tot
            tot += R[w][c]
    totcols = tot

    nex = 2 + len(extra)
    exnames = sorted(extra.keys())
    idx_wrapped = []
    segv = []
    for k in range(NC):
        idxk = np.zeros((totcols, 128), dtype=np.int16)
        vals = np.zeros((nex, totcols, 128), dtype=np.float32)
        for w in range(NWIN):
            for c in range(nchunk):
                g = (k * NWIN + w) * nchunk + c
                s0 = starts[g]
                f = F[w][c]
                for j, (ptr, take) in enumerate(packs[(k, w, c)]):
                    if take <= 0:
                        continue
                    col = col_off[w][c] + j
                    sl = slice(s0 + ptr, s0 + ptr + take)
                    idxk[col, :take] = tab_all[sl].astype(np.int16)
                    vals[0, col, :take] = (segs_all[sl] - f[j]).astype(np.float32)
                    vals[1, col, :take] = invdst_all[sl]
                    for ei, en in enumerate(exnames):
                        vals[2 + ei, col, :take] = extra_all[en][sl]
        iw = idxk.reshape(totcols, 8, 16).transpose(2, 0, 1).reshape(16, totcols * 8)
        idx_wrapped.append(np.tile(iw, (8, 1)))
        segv.append(np.ascontiguousarray(vals.transpose(2, 0, 1)))  # [128, nex, totcols]

    return dict(R=R, F=F, col_off=col_off, totcols=totcols, nex=nex,
                idx=idx_wrapped, segv=segv, span_of=span_of)


def plan(cfg, src, dst):
    """Host planning for both gather passes."""
    chunk1 = src // cfg.CHUNK
    tab1 = (src - chunk1 * cfg.CHUNK).astype(np.int64)
    p1 = _plan_pass(cfg, dst, None, 0, cfg.NCHUNK, chunk1, tab1, {})

    # layer 2: gather 64-value blocks of the padded s table
    kc = src // cfg.NPC
    padrow = kc * cfg.SPAD + (src - kc * cfg.NPC)
    blk = padrow // 64
    off = (padrow - blk * 64).astype(np.float32)
    assert blk.max() < cfg.NB2 <= 32767
    p2 = _plan_pass(cfg, dst, None, 0, 1, np.zeros_like(src), blk,
                    {"off": off})
    return p1, p2


def pack_weights(cfg, fc_W, fc_b, Wl1, bl1, Wr1, Wl2, bl2, Wr2, head_W, head_b):
    """One [128, WCOLS] f32 tensor with all constants."""
    D, H, SEGW = cfg.D, cfg.H, cfg.SEGW
    alpha = (head_W @ Wl2)[0]            # [H]
    beta = (head_W @ Wr2)[0]             # [H]
    c0 = float(head_b[0] + head_W[0] @ bl2)

    cols = {}
    seg = []

    def add(name, arr):
        arr = np.asarray(arr, np.float32)
        assert arr.ndim == 2 and arr.shape[0] <= 128
        cols[name] = sum(a.shape[1] for a in seg)
        pad = np.zeros((128, arr.shape[1]), np.float32)
        pad[:arr.shape[0]] = arr
        seg.append(pad)

    add("ident", np.eye(D, dtype=np.float32))
    add("iota32", np.tile(np.arange(SEGW, dtype=np.float32), (128, 1)))
    add("iota64", np.tile(np.arange(64, dtype=np.float32), (128, 1)))
    add("fcWT", fc_W.T)                  # [D, H]
    add("Wl1T", Wl1.T)                   # [H, H]
    add("Wr1T", Wr1.T)
    add("alpha", alpha[:, None])         # [H, 1]
    add("beta", beta[:, None])
    add("bl1", bl1[:, None])
    add("fcb", fc_b[:, None])
    add("ones", np.ones((H, 1), np.float32))
    add("c0", np.full((1, 1), c0, np.float32))
    wt = np.concatenate(seg, axis=1)
    return wt, cols


def build(cfg, p1, p2, wcols, wcols_n):
    N, D, H, NPC, WIN, NWIN = cfg.N, cfg.D, cfg.H, cfg.NPC, cfg.WIN, cfg.NWIN
    NCHUNK, CHUNK, SEGW, COLB, XB = cfg.NCHUNK, cfg.CHUNK, cfg.SEGW, cfg.COLB, cfg.XB
    NBLK, SPAD, NB2, GB2 = cfg.NBLK, cfg.SPAD, cfg.NB2, cfg.GB2
    R1, F1, co1, tc1 = p1["R"], p1["F"], p1["col_off"], p1["totcols"]
    R2, F2, co2, tc2 = p2["R"], p2["F"], p2["col_off"], p2["totcols"]
    span_of = p1["span_of"]

    nc = bacc.Bacc(None, target_bir_lowering=False, num_swdge_queues=NSWQ,
                   num_devices=cfg.NC)
    wt_d = nc.dram_tensor("wt", [128, wcols_n], F32, kind="ExternalInput")
    x_d = nc.dram_tensor("x_my", [NPC, D], F32, kind="ExternalInput")
    idx1_d = nc.dram_tensor("idx1", [128, tc1 * 8], I16, kind="ExternalInput")
    segv1_d = nc.dram_tensor("segv1", [128, 2, tc1], F32, kind="ExternalInput")
    idx2_d = nc.dram_tensor("idx2", [128, tc2 * 8], I16, kind="ExternalInput")
    segv2_d = nc.dram_tensor("segv2", [128, 3, tc2], F32, kind="ExternalInput")
    out_d = nc.dram_tensor("out", [1, NPC], F32, kind="ExternalOutput")

    rg = [list(range(cfg.NC))]

    with TileContext(nc) as tc:
        with (
            tc.tile_pool(name="dram", bufs=1, space="DRAM") as dramp,
            tc.tile_pool(name="const", bufs=1) as constp,
            tc.tile_pool(name="resid", bufs=1) as resid,
            tc.tile_pool(name="io", bufs=3) as iop,
            tc.tile_pool(name="gat", bufs=2) as gatp,
            tc.tile_pool(name="bb", bufs=3) as bbp,
            tc.tile_pool(name="ps", bufs=4, space="PSUM") as psp,
            tc.tile_pool(name="ps2", bufs=2, space="PSUM") as psp2,
        ):
            ag_in = dramp.tile([NPC, H], F32)
            h0_full = dramp.tile([N, H], F32, addr_space="Shared")
            hT_dram = dramp.tile([H, NPC], F32)
            t_dram = dramp.tile([1, NPC], F32)
            s_ag_in = dramp.tile([SPAD, 1], F32)
            s_full = dramp.tile([NB2, 64], F32, addr_space="Shared")

            wt = constp.tile([128, wcols_n], F32)
            nc.sync.dma_start(wt[:], wt_d[:, :])
            ident = wt[:, wcols["ident"]:wcols["ident"] + D]
            iota32 = wt[:, wcols["iota32"]:wcols["iota32"] + SEGW]
            iota64 = wt[:, wcols["iota64"]:wcols["iota64"] + 64]
            fcWT = wt[:D, wcols["fcWT"]:wcols["fcWT"] + H]
            Wl1T = wt[:H, wcols["Wl1T"]:wcols["Wl1T"] + H]
            Wr1T = wt[:H, wcols["Wr1T"]:wcols["Wr1T"] + H]
            alpha = wt[:H, wcols["alpha"]:wcols["alpha"] + 1]
            beta = wt[:H, wcols["beta"]:wcols["beta"] + 1]
            bl1 = wt[:H, wcols["bl1"]:wcols["bl1"] + 1]
            fcb = wt[:H, wcols["fcb"]:wcols["fcb"] + 1]
            ones = wt[:H, wcols["ones"]:wcols["ones"] + 1]
            c0 = wt[0:1, wcols["c0"]:wcols["c0"] + 1]

            # s values, block-column layout: [p, b] = s[node b*128+p]
            s_sb = resid.tile([128, NBLK], F32)
            nc.vector.memset(s_sb[:, :], 0.0)

            # ---- fc stage over my shard: h0 rows -> ag_in, h0T -> hT_dram --
            nt = math.ceil(NPC / 128)
            for i0 in range(0, nt, XB):
                nb = min(XB, nt - i0)
                rows0 = i0 * 128
                rowsn = min(NPC - rows0, nb * 128)
                xb_t = iop.tile([128, XB, D], F32, tag="xb")
                nfull = rowsn // 128
                if nfull:
                    nc.sync.dma_start(
                        xb_t[:, :nfull, :],
                        x_d[rows0:rows0 + nfull * 128, :].rearrange(
                            "(t p) d -> p t d", p=128))
                rem = rowsn - nfull * 128
                if rem:
                    nc.sync.dma_start(xb_t[:rem, nfull, :],
                                      x_d[rows0 + nfull * 128:rows0 + rowsn, :])
                hb_t = iop.tile([128, XB, H], F32, tag="hb")
                hTb_t = iop.tile([H, XB, 128], F32, tag="hTb")
                for t in range(nb):
                    rows = min(128, NPC - rows0 - t * 128)
                    xT_ps = psp.tile([128, D], F32, space="PSUM", tag="s")
                    nc.tensor.transpose(xT_ps[:, :rows], xb_t[:rows, t, :],
                                        ident[:rows, :rows])
                    xT = iop.tile([128, D], F32, tag="xT_sb")
                    nc.vector.tensor_copy(xT[:], xT_ps[:])
                    hT_ps = psp.tile([H, 128], F32, space="PSUM", tag="s")
                    nc.tensor.matmul(out=hT_ps[:, :rows], lhsT=fcWT, rhs=xT[:, :rows],
                                     start=True, stop=True)
                    nc.scalar.activation(hTb_t[:, t, :rows], hT_ps[:, :rows],
                                         mybir.ActivationFunctionType.Relu, bias=fcb)
                    h_ps = psp.tile([128, H], F32, space="PSUM", tag="s")
                    nc.tensor.transpose(h_ps[:rows, :], hTb_t[:, t, :rows],
                                        ident[:H, :H])
                    nc.vector.tensor_copy(hb_t[:rows, t, :], h_ps[:rows, :])
                nc.sync.dma_start(hT_dram[:, rows0:rows0 + rowsn],
                                  hTb_t[:, :nb, :].rearrange(
                                      "h t p -> h (t p)")[:, :rowsn])
                if nfull:
                    nc.sync.dma_start(
                        ag_in[rows0:rows0 + nfull * 128, :].rearrange(
                            "(t p) d -> p t d", p=128),
                        hb_t[:, :nfull, :])
                if rem:
                    nc.sync.dma_start(ag_in[rows0 + nfull * 128:rows0 + rowsn, :],
                                      hb_t[:rem, nfull, :])

            nc.gpsimd.collective_compute(
                "AllGather", mybir.AluOpType.bypass, replica_groups=rg,
                ins=[ag_in.opt()], outs=[h0_full.opt()])

            # ---- layer-1 window loop -----------------------------------
            for w in range(NWIN):
                span = span_of(w)
                cw0 = co1[w][0]
                cwn = co1[w][NCHUNK - 1] + R1[w][NCHUNK - 1] - cw0

                idx_w = iop.tile([128, cwn * 8], I16, tag="idxw")
                nc.sync.dma_start(idx_w[:], idx1_d[:, cw0 * 8:(cw0 + cwn) * 8])
                segv_w = iop.tile([128, 2, cwn], F32, tag="segvw")
                nc.sync.dma_start(segv_w[:], segv1_d[:, :, cw0:cw0 + cwn])
                hTw = iop.tile([H, WIN], F32, tag="hTw")
                nc.sync.dma_start(hTw[:, :span],
                                  hT_dram[:, w * WIN:w * WIN + span])

                agg_ps = psp2.tile([H, WIN], F32, space="PSUM", tag="agg")
                nc.vector.memset(agg_ps[:, :], 0.0)

                mm = []
                for c in range(NCHUNK):
                    r = R1[w][c]
                    o = co1[w][c] - cw0
                    wg = gatp.tile([128, r, H], F32, tag=f"wg{c % 2}")
                    nc.gpsimd.dma_gather(
                        out_ap=wg[:, :, :],
                        in_ap=h0_full[c * CHUNK:min((c + 1) * CHUNK, N), :],
                        idxs_ap=idx_w[:, o * 8:(o + r) * 8],
                        num_idxs=r * 128, num_idxs_reg=r * 128, elem_size=H,
                        single_packet=False, queue_num=c % NSWQ)
                    for b0 in range(0, r, COLB):
                        nb = min(COLB, r - b0)
                        B = bbp.tile([128, COLB, SEGW], F32, tag="B")
                        seg_b = segv_w[:, 0, o + b0:o + b0 + nb].unsqueeze(2) \
                            .to_broadcast([128, nb, SEGW])
                        v_b = segv_w[:, 1, o + b0:o + b0 + nb].unsqueeze(2) \
                            .to_broadcast([128, nb, SEGW])
                        iota_b = iota32.unsqueeze(1).to_broadcast([128, nb, SEGW])
                        nc.vector.tensor_tensor(out=B[:, :nb, :], in0=seg_b, in1=iota_b,
                                                op=mybir.AluOpType.is_equal)
                        nc.vector.tensor_tensor(out=B[:, :nb, :], in0=B[:, :nb, :],
                                                in1=v_b, op=mybir.AluOpType.mult)
                        for t in range(nb):
                            j = b0 + t
                            f = F1[w][c][j]
                            sw = min(SEGW, span - f)
                            mm.append((wg, j, B, t, f, sw))
                for q, (wg, j, B, t, f, sw) in enumerate(mm):
                    nc.tensor.matmul(out=agg_ps[:, f:f + sw], lhsT=wg[:, j, :],
                                     rhs=B[:, t, :sw], start=False,
                                     stop=(q == len(mm) - 1), skip_group_check=True)

                aggT = iop.tile([H, WIN], F32, tag="aggT")
                nc.scalar.copy(aggT[:, :span], agg_ps[:, :span])

                z_ps = psp2.tile([H, WIN], F32, space="PSUM", tag="z")
                nc.tensor.matmul(out=z_ps[:, :span], lhsT=Wl1T, rhs=aggT[:, :span],
                                 start=True, stop=False)
                nc.tensor.matmul(out=z_ps[:, :span], lhsT=Wr1T,
                                 rhs=hTw[:, :span], start=False, stop=True)

                h1T_sb = iop.tile([H, WIN], F32, tag="h1T")
                nc.scalar.activation(h1T_sb[:, :span], z_ps[:, :span],
                                     mybir.ActivationFunctionType.Relu, bias=bl1)

                # s per 128-node block (column layout), t as row
                for b in range(math.ceil(span / 128)):
                    rows = min(128, span - b * 128)
                    s_ps = psp.tile([128, 1], F32, space="PSUM", tag="s")
                    nc.tensor.matmul(out=s_ps[:rows, :],
                                     lhsT=h1T_sb[:, b * 128:b * 128 + rows],
                                     rhs=alpha, start=True, stop=True)
                    nc.scalar.copy(s_sb[:rows, w * 4 + b:w * 4 + b + 1],
                                   s_ps[:rows, :])
                t_ps = psp.tile([1, WIN], F32, space="PSUM", tag="s")
                nc.tensor.matmul(out=t_ps[:, :span], lhsT=beta,
                                 rhs=h1T_sb[:, :span], start=True, stop=True)
                t_sb = iop.tile([1, WIN], F32, tag="tsb")
                nc.scalar.activation(t_sb[:, :span], t_ps[:, :span],
                                     mybir.ActivationFunctionType.Identity,
                                     bias=c0)
                nc.sync.dma_start(t_dram[:, w * WIN:w * WIN + span],
                                  t_sb[:, :span])

            # ---- s AllGather -------------------------------------------
            nc.sync.dma_start(
                s_ag_in[:, :].rearrange("(t p) o -> p (t o)", p=128), s_sb[:, :])
            nc.gpsimd.collective_compute(
                "AllGather", mybir.AluOpType.bypass, replica_groups=rg,
                ins=[s_ag_in.opt()], outs=[s_full.opt()])

            # ---- layer-2 window loop (scalar messages) ------------------
            for w in range(NWIN):
                span = span_of(w)
                cw0 = co2[w][0]
                cwn = R2[w][0]

                idx2_w = iop.tile([128, cwn * 8], I16, tag="idx2w")
                nc.sync.dma_start(idx2_w[:], idx2_d[:, cw0 * 8:(cw0 + cwn) * 8])
                segv2_w = iop.tile([128, 3, cwn], F32, tag="segv2w")
                nc.sync.dma_start(segv2_w[:], segv2_d[:, :, cw0:cw0 + cwn])
                t_row = iop.tile([1, WIN], F32, tag="trow")
                nc.sync.dma_start(t_row[:, :span],
                                  t_dram[:, w * WIN:w * WIN + span])

                agg2_ps = psp2.tile([H, WIN], F32, space="PSUM", tag="agg")
                nc.vector.memset(agg2_ps[:, :], 0.0)

                mm = []
                for g0 in range(0, cwn, GB2):
                    gn = min(GB2, cwn - g0)
                    wg2 = gatp.tile([128, GB2, 64], F32, tag=f"wg2{(g0 // GB2) % 2}")
                    nc.gpsimd.dma_gather(
                        out_ap=wg2[:, :gn, :], in_ap=s_full[:, :],
                        idxs_ap=idx2_w[:, (g0) * 8:(g0 + gn) * 8],
                        num_idxs=gn * 128, num_idxs_reg=gn * 128, elem_size=64,
                        single_packet=False, queue_num=(g0 // GB2) % NSWQ)
                    for b0 in range(g0, g0 + gn, COLB):
                        nb = min(COLB, g0 + gn - b0)
                        # select mask: one-hot over 64 at off, scaled later by B2
                        SEL = bbp.tile([128, COLB, 64], F32, tag="SEL")
                        off_b = segv2_w[:, 2, b0:b0 + nb].unsqueeze(2) \
                            .to_broadcast([128, nb, 64])
                        iota_b = iota64.unsqueeze(1).to_broadcast([128, nb, 64])
                        nc.vector.tensor_tensor(out=SEL[:, :nb, :], in0=off_b,
                                                in1=iota_b,
                                                op=mybir.AluOpType.is_equal)
                        nc.vector.tensor_tensor(
                            out=wg2[:, b0 - g0:b0 - g0 + nb, :],
                            in0=wg2[:, b0 - g0:b0 - g0 + nb, :],
                            in1=SEL[:, :nb, :], op=mybir.AluOpType.mult)
                        B = bbp.tile([128, COLB, SEGW], F32, tag="B2")
                        seg_b = segv2_w[:, 0, b0:b0 + nb].unsqueeze(2) \
                            .to_broadcast([128, nb, SEGW])
                        v_b = segv2_w[:, 1, b0:b0 + nb].unsqueeze(2) \
                            .to_broadcast([128, nb, SEGW])
                        iotas_b = iota32.unsqueeze(1).to_broadcast([128, nb, SEGW])
                        nc.vector.tensor_tensor(out=B[:, :nb, :], in0=seg_b,
                                                in1=iotas_b,
                                                op=mybir.AluOpType.is_equal)
                        nc.vector.tensor_tensor(out=B[:, :nb, :], in0=B[:, :nb, :],
                                                in1=v_b, op=mybir.AluOpType.mult)
                        for t in range(nb):
                            j = b0 + t
                            f = F2[w][0][j]
                            sw = min(SEGW, span - f)
                            mm.append((wg2, j - g0, B, t, f, sw))
                for q, (wg2, j, B, t, f, sw) in enumerate(mm):
                    nc.tensor.matmul(out=agg2_ps[:, f:f + sw], lhsT=wg2[:, j, :],
                                     rhs=B[:, t, :sw], start=False,
                                     stop=(q == len(mm) - 1), skip_group_check=True)

                agg2T = iop.tile([H, WIN], F32, tag="aggT")
                nc.scalar.copy(agg2T[:, :span], agg2_ps[:, :span])
                o_ps = psp.tile([1, WIN], F32, space="PSUM", tag="s")
                nc.tensor.matmul(out=o_ps[:, :span], lhsT=ones,
                                 rhs=agg2T[:, :span], start=True, stop=True)
                out_sb = iop.tile([1, WIN], F32, tag="outsb")
                nc.vector.tensor_tensor(out=out_sb[:, :span], in0=o_ps[:, :span],
                                        in1=t_row[:, :span],
                                        op=mybir.AluOpType.add)
                nc.sync.dma_start(out_d[:, w * WIN:w * WIN + span],
                                  out_sb[:, :span])

    nc.compile()
    return nc


def kernel(x, edge_index, batch, fc_W, fc_b, Wl1, bl1, Wr1, Wl2, bl2, Wr2,
           head_W, head_b, cfg=None):
    cfg = cfg or Cfg(n_nodes=x.shape[0], in_dim=x.shape[1], hid=fc_W.shape[0])
    x = np.asarray(x, dtype=np.float32)
    src = np.asarray(edge_index[0], dtype=np.int64)
    dst = np.asarray(edge_index[1], dtype=np.int64)
    fc_W, fc_b, Wl1, bl1, Wr1, Wl2, bl2, Wr2, head_W, head_b = [
        np.asarray(a, dtype=np.float32)
        for a in (fc_W, fc_b, Wl1, bl1, Wr1, Wl2, bl2, Wr2, head_W, head_b)]

    t0 = time.time()
    p1, p2 = plan(cfg, src, dst)
    timings["plan_s"] = time.time() - t0
    wt, wcols = pack_weights(cfg, fc_W, fc_b, Wl1, bl1, Wr1,
                             Wl2, bl2, Wr2, head_W, head_b)

    t0 = time.time()
    ncb = build(cfg, p1, p2, wcols, wt.shape[1])
    timings["build_s"] = time.time() - t0

    in_maps = []
    for k in range(cfg.NC):
        in_maps.append({
            "wt": wt, "idx1": p1["idx"][k], "segv1": p1["segv"][k],
            "idx2": p2["idx"][k], "segv2": p2["segv"][k],
            "x_my": x[k * cfg.NPC:(k + 1) * cfg.NPC],
        })
    t0 = time.time()
    res = run_bass_kernel_spmd(ncb, in_maps, core_ids=list(range(cfg.NC)))
    timings["run_s"] = time.time() - t0
    timings["hw_ns"] = res.exec_time_ns
    out = np.concatenate([r["out"][0] for r in res.results], axis=0)
    return out.reshape(cfg.N, 1).astype(np.float32)


# revision 5
# speedup vs baseline: 1.9427x; 1.9427x over previous
"""Trainium2 Bass kernel for CustomGNN (fc+relu -> SAGEConv+relu -> SAGEConv -> head).

Single-NEFF SPMD design (8 NeuronCores, one program, per-core data):
  - nodes sharded in 8 contiguous chunks of 12500 (dst-partitioned edges).
  - fc stage sharded: each core computes h0 for its own nodes, then an
    AllGather builds the full [100000, 64] h0 table in every core's HBM.
  - SAGE layer 1: per 512-dst window, edges (sorted by dst, packed into
    128-slot columns) gather h0[src] rows (256B) with dma_gather from 4 src
    chunks (int16 index range); segment-sum via PE matmuls against one-hot
    inv-deg matrices built on DVE (is_equal vs iota over 32-wide sliding seg
    windows); then zT = Wl1 @ aggT + Wr1 @ h0T + bl1, relu -> h1T.
  - SAGE layer 2 + head collapsed to scalars (linearity of the head):
      out[v] = invdeg_v * sum_{u->v} s[u] + t[v]
      s[u] = h1[u] . (head_W @ Wl2),  t[v] = h1[v] . (head_W @ Wr2) + c0
    s is AllGathered (50KB/rank, padded shard stride 12544), then a second
    window pass gathers 64-value blocks of the dense s table (idx = padded
    row >> 6, fits int16 with no chunking), selects the scalar with a DVE
    one-hot over 64, and folds the 64-wide reduction into the PE
    segment-sum matmuls (final ones-vector partition reduce per window).

All index-derived structure (column quotas R, seg-window starts F) is computed
on the host from edge_index and baked into the program; quotas are max-merged
across the 8 cores so the SPMD program is identical on every core.
"""
import math
import time
import numpy as np

from concourse import bacc, mybir
from concourse.tile import TileContext
from concourse.bass_utils import run_bass_kernel_spmd

timings = {}

NSWQ = 2

F32 = mybir.dt.float32
I16 = mybir.dt.int16


class Cfg:
    def __init__(self, n_nodes=100000, in_dim=128, hid=64, ncores=8,
                 win=512, nchunk=4, segw=32, colb=8, xb=4, gb2=48):
        assert n_nodes % ncores == 0
        self.N = n_nodes
        self.D = in_dim
        self.H = hid
        self.NC = ncores
        self.NPC = n_nodes // ncores
        self.WIN = win
        self.NWIN = math.ceil(self.NPC / win)
        self.NCHUNK = nchunk
        self.CHUNK = math.ceil(n_nodes / nchunk)
        assert self.CHUNK <= 32767
        self.SEGW = segw
        self.COLB = colb
        self.XB = xb
        self.GB2 = gb2                        # layer-2 gather block (columns)
        self.NBLK = math.ceil(self.NPC / 128)
        self.SPAD = self.NBLK * 128           # padded shard stride (12544)
        assert self.SPAD % 64 == 0
        self.NB2 = self.SPAD * ncores // 64   # layer-2 table rows ([NB2, 64])


def _schedule_F(R, span, segw):
    if R <= 1:
        return [0] * max(R, 1)
    top = max(span - segw, 0)
    return [min((j * top) // (R - 1), top) for j in range(R)]


def _schedule_F_adaptive(segs_by_core, r, span, segw):
    """Place col windows at worst-case rank positions across cores: col j
    targets rank ~j*128; window start bounded above by every core's seg at
    that rank (no lagging edge) and nudged up to cover the window tail."""
    top = max(span - segw, 0)
    nmax = max((len(s) for s in segs_by_core), default=0)
    if r <= 1 or nmax == 0:
        return [0] * max(r, 1)
    F = []
    prev = 0
    for j in range(r):
        m = (j * (nmax - 1)) // (r - 1)
        hi = top
        lo = 0
        for s in segs_by_core:
            n = len(s)
            if n:
                hi = min(hi, int(s[min(m, n - 1)]))
                lo = max(lo, int(s[min(m + 127, n - 1)]) - (segw - 1))
        f = max(prev, min(hi, top))
        f = max(f, min(lo, top))
        F.append(f)
        prev = f
    return F


def _pack_group(segs, R, F, segw):
    """Greedy-pack sorted segs into R cols of <=128 edges, col j window
    [F[j], F[j]+segw). Returns list of (start, count) per col, or None."""
    n = len(segs)
    ptr = 0
    out = []
    for j in range(R):
        if ptr < n and segs[ptr] < F[j]:
            return None
        hi = np.searchsorted(segs, F[j] + segw, side="left")
        take = min(128, hi - ptr)
        out.append((ptr, take))
        ptr += take
    if ptr != n:
        return None
    return out


def _plan_pass(cfg, dst, seg_all_src, gid_base, nchunk, chunk, tabidx, extra):
    """Generic planning for one gather+segment-sum pass.

    dst: global dst per edge; chunk: chunk id per edge (grouping + gather
    table split); tabidx: per-edge gather index (relative to chunk base
    handled by caller); extra: dict name -> per-edge f32 array baked beside
    (seg, invdeg) into the segv tensor.
    Returns structure dict + per-core idx (int16, wrapped) and segv arrays.
    """
    N, NC, NPC, WIN, NWIN, SEGW = cfg.N, cfg.NC, cfg.NPC, cfg.WIN, cfg.NWIN, cfg.SEGW

    deg = np.bincount(dst, minlength=N)
    inv = (1.0 / np.maximum(deg, 1)).astype(np.float32)

    core = dst // NPC
    local = dst - core * NPC
    win = local // WIN
    seg = (local - win * WIN).astype(np.int32)

    gid = ((core * NWIN + win) * nchunk + chunk).astype(np.int64)
    order = np.lexsort((seg, gid))
    gids = gid[order]
    segs_all = seg[order]
    tab_all = tabidx[order]
    invdst_all = inv[dst[order]].astype(np.float32)
    extra_all = {k: v[order] for k, v in extra.items()}

    ngroups = NC * NWIN * nchunk
    counts = np.bincount(gids, minlength=ngroups)
    starts = np.zeros(ngroups + 1, dtype=np.int64)
    np.cumsum(counts, out=starts[1:])

    def span_of(w):
        return min(WIN, NPC - w * WIN)

    R = [[0] * nchunk for _ in range(NWIN)]
    F = [[None] * nchunk for _ in range(NWIN)]
    packs = {}
    for w in range(NWIN):
        span = span_of(w)
        for c in range(nchunk):
            r = 1
            for k in range(NC):
                g = (k * NWIN + w) * nchunk + c
                r = max(r, math.ceil(max(counts[g], 1) / 128))
            while True:
                f = _schedule_F(r, span, SEGW)
                ok = True
                for k in range(NC):
                    g = (k * NWIN + w) * nchunk + c
                    segs = segs_all[starts[g]:starts[g + 1]]
                    p = _pack_group(segs, r, f, SEGW)
                    if p is None:
                        ok = False
                        break
                    packs[(k, w, c)] = p
                if ok:
                    break
                r += 1
            R[w][c] = r
            F[w][c] = f

    col_off = [[0] * nchunk for _ in range(NWIN)]
    tot = 0
    for w in range(NWIN):
        for c in range(nchunk):
            col_off[w][c] = tot
            tot += R[w][c]
    totcols = tot

    nex = 2 + len(extra)
    exnames = sorted(extra.keys())
    idx_wrapped = []
    segv = []
    for k in range(NC):
        idxk = np.zeros((totcols, 128), dtype=np.int16)
        vals = np.zeros((nex, totcols, 128), dtype=np.float32)
        for w in range(NWIN):
            for c in range(nchunk):
                g = (k * NWIN + w) * nchunk + c
                s0 = starts[g]
                f = F[w][c]
                for j, (ptr, take) in enumerate(packs[(k, w, c)]):
                    if take <= 0:
                        continue
                    col = col_off[w][c] + j
                    sl = slice(s0 + ptr, s0 + ptr + take)
                    idxk[col, :take] = tab_all[sl].astype(np.int16)
                    vals[0, col, :take] = (segs_all[sl] - f[j]).astype(np.float32)
                    vals[1, col, :take] = invdst_all[sl]
                    for ei, en in enumerate(exnames):
                        vals[2 + ei, col, :take] = extra_all[en][sl]
        iw = idxk.reshape(totcols, 8, 16).transpose(2, 0, 1).reshape(16, totcols * 8)
        idx_wrapped.append(np.tile(iw, (8, 1)))
        segv.append(np.ascontiguousarray(vals.transpose(2, 0, 1)))  # [128, nex, totcols]

    return dict(R=R, F=F, col_off=col_off, totcols=totcols, nex=nex,
                idx=idx_wrapped, segv=segv, span_of=span_of)


def plan(cfg, src, dst):
    """Host planning for both gather passes."""
    chunk1 = src // cfg.CHUNK
    tab1 = (src - chunk1 * cfg.CHUNK).astype(np.int64)
    p1 = _plan_pass(cfg, dst, None, 0, cfg.NCHUNK, chunk1, tab1, {})

    # layer 2: gather 64-value blocks of the padded s table
    kc = src // cfg.NPC
    padrow = kc * cfg.SPAD + (src - kc * cfg.NPC)
    blk = padrow // 64
    off = (padrow - blk * 64).astype(np.float32)
    assert blk.max() < cfg.NB2 <= 32767
    p2 = _plan_pass(cfg, dst, None, 0, 1, np.zeros_like(src), blk,
                    {"off": off})
    return p1, p2


def pack_weights(cfg, fc_W, fc_b, Wl1, bl1, Wr1, Wl2, bl2, Wr2, head_W, head_b):
    """One [128, WCOLS] f32 tensor with all constants."""
    D, H, SEGW = cfg.D, cfg.H, cfg.SEGW
    alpha = (head_W @ Wl2)[0]            # [H]
    beta = (head_W @ Wr2)[0]             # [H]
    c0 = float(head_b[0] + head_W[0] @ bl2)

    cols = {}
    seg = []

    def add(name, arr):
        arr = np.asarray(arr, np.float32)
        assert arr.ndim == 2 and arr.shape[0] <= 128
        cols[name] = sum(a.shape[1] for a in seg)
        pad = np.zeros((128, arr.shape[1]), np.float32)
        pad[:arr.shape[0]] = arr
        seg.append(pad)

    add("ident", np.eye(D, dtype=np.float32))
    add("iota32", np.tile(np.arange(SEGW, dtype=np.float32), (128, 1)))
    add("iota64", np.tile(np.arange(64, dtype=np.float32), (128, 1)))
    add("fcWT", fc_W.T)                  # [D, H]
    add("Wl1T", Wl1.T)                   # [H, H]
    add("Wr1T", Wr1.T)
    add("alpha", alpha[:, None])         # [H, 1]
    add("beta", beta[:, None])
    add("bl1", bl1[:, None])
    add("fcb", fc_b[:, None])
    add("ones", np.ones((H, 1), np.float32))
    add("c0", np.full((1, 1), c0, np.float32))
    wt = np.concatenate(seg, axis=1)
    return wt, cols


def build(cfg, p1, p2, wcols, wcols_n):
    N, D, H, NPC, WIN, NWIN = cfg.N, cfg.D, cfg.H, cfg.NPC, cfg.WIN, cfg.NWIN
    NCHUNK, CHUNK, SEGW, COLB, XB = cfg.NCHUNK, cfg.CHUNK, cfg.SEGW, cfg.COLB, cfg.XB
    NBLK, SPAD, NB2, GB2 = cfg.NBLK, cfg.SPAD, cfg.NB2, cfg.GB2
    R1, F1, co1, tc1 = p1["R"], p1["F"], p1["col_off"], p1["totcols"]
    R2, F2, co2, tc2 = p2["R"], p2["F"], p2["col_off"], p2["totcols"]
    span_of = p1["span_of"]

    nc = bacc.Bacc(None, target_bir_lowering=False, num_swdge_queues=NSWQ,
                   num_devices=cfg.NC)
    wt_d = nc.dram_tensor("wt", [128, wcols_n], F32, kind="ExternalInput")
    x_d = nc.dram_tensor("x_my", [NPC, D], F32, kind="ExternalInput")
    idx1_d = nc.dram_tensor("idx1", [128, tc1 * 8], I16, kind="ExternalInput")
    segv1_d = nc.dram_tensor("segv1", [128, 2, tc1], F32, kind="ExternalInput")
    idx2_d = nc.dram_tensor("idx2", [128, tc2 * 8], I16, kind="ExternalInput")
    segv2_d = nc.dram_tensor("segv2", [128, 3, tc2], F32, kind="ExternalInput")
    out_d = nc.dram_tensor("out", [1, NPC], F32, kind="ExternalOutput")

    rg = [list(range(cfg.NC))]

    with TileContext(nc) as tc:
        with (
            tc.tile_pool(name="dram", bufs=1, space="DRAM") as dramp,
            tc.tile_pool(name="const", bufs=1) as constp,
            tc.tile_pool(name="resid", bufs=1) as resid,
            tc.tile_pool(name="io", bufs=3) as iop,
            tc.tile_pool(name="gat", bufs=2) as gatp,
            tc.tile_pool(name="bb", bufs=3) as bbp,
            tc.tile_pool(name="ps", bufs=4, space="PSUM") as psp,
            tc.tile_pool(name="ps2", bufs=2, space="PSUM") as psp2,
        ):
            ag_in = dramp.tile([NPC, H], F32)
            h0_full = dramp.tile([N, H], F32, addr_space="Shared")
            hT_dram = dramp.tile([H, NPC], F32)
            t_dram = dramp.tile([1, NPC], F32)
            s_ag_in = dramp.tile([SPAD, 1], F32)
            s_full = dramp.tile([NB2, 64], F32, addr_space="Shared")

            wt = constp.tile([128, wcols_n], F32)
            nc.sync.dma_start(wt[:], wt_d[:, :])
            ident = wt[:, wcols["ident"]:wcols["ident"] + D]
            iota32 = wt[:, wcols["iota32"]:wcols["iota32"] + SEGW]
            iota64 = wt[:, wcols["iota64"]:wcols["iota64"] + 64]
            fcWT = wt[:D, wcols["fcWT"]:wcols["fcWT"] + H]
            Wl1T = wt[:H, wcols["Wl1T"]:wcols["Wl1T"] + H]
            Wr1T = wt[:H, wcols["Wr1T"]:wcols["Wr1T"] + H]
            alpha = wt[:H, wcols["alpha"]:wcols["alpha"] + 1]
            beta = wt[:H, wcols["beta"]:wcols["beta"] + 1]
            bl1 = wt[:H, wcols["bl1"]:wcols["bl1"] + 1]
            fcb = wt[:H, wcols["fcb"]:wcols["fcb"] + 1]
            ones = wt[:H, wcols["ones"]:wcols["ones"] + 1]
            c0 = wt[0:1, wcols["c0"]:wcols["c0"] + 1]

            # s values, block-column layout: [p, b] = s[node b*128+p]
            s_sb = resid.tile([128, NBLK], F32)
            nc.vector.memset(s_sb[:, :], 0.0)

            # ---- fc stage over my shard: h0 rows -> ag_in, h0T -> hT_dram --
            nt = math.ceil(NPC / 128)
            for i0 in range(0, nt, XB):
                nb = min(XB, nt - i0)
                rows0 = i0 * 128
                rowsn = min(NPC - rows0, nb * 128)
                xb_t = iop.tile([128, XB, D], F32, tag="xb")
                nfull = rowsn // 128
                if nfull:
                    nc.sync.dma_start(
                        xb_t[:, :nfull, :],
                        x_d[rows0:rows0 + nfull * 128, :].rearrange(
                            "(t p) d -> p t d", p=128))
                rem = rowsn - nfull * 128
                if rem:
                    nc.sync.dma_start(xb_t[:rem, nfull, :],
                                      x_d[rows0 + nfull * 128:rows0 + rowsn, :])
                hb_t = iop.tile([128, XB, H], F32, tag="hb")
                hTb_t = iop.tile([H, XB, 128], F32, tag="hTb")
                for t in range(nb):
                    rows = min(128, NPC - rows0 - t * 128)
                    xT_ps = psp.tile([128, D], F32, space="PSUM", tag="s")
                    nc.tensor.transpose(xT_ps[:, :rows], xb_t[:rows, t, :],
                                        ident[:rows, :rows])
                    xT = iop.tile([128, D], F32, tag="xT_sb")
                    nc.vector.tensor_copy(xT[:], xT_ps[:])
                    hT_ps = psp.tile([H, 128], F32, space="PSUM", tag="s")
                    nc.tensor.matmul(out=hT_ps[:, :rows], lhsT=fcWT, rhs=xT[:, :rows],
                                     start=True, stop=True)
                    nc.scalar.activation(hTb_t[:, t, :rows], hT_ps[:, :rows],
                                         mybir.ActivationFunctionType.Relu, bias=fcb)
                    h_ps = psp.tile([128, H], F32, space="PSUM", tag="s")
                    nc.tensor.transpose(h_ps[:rows, :], hTb_t[:, t, :rows],
                                        ident[:H, :H])
                    nc.vector.tensor_copy(hb_t[:rows, t, :], h_ps[:rows, :])
                nc.sync.dma_start(hT_dram[:, rows0:rows0 + rowsn],
                                  hTb_t[:, :nb, :].rearrange(
                                      "h t p -> h (t p)")[:, :rowsn])
                if nfull:
                    nc.sync.dma_start(
                        ag_in[rows0:rows0 + nfull * 128, :].rearrange(
                            "(t p) d -> p t d", p=128),
                        hb_t[:, :nfull, :])
                if rem:
                    nc.sync.dma_start(ag_in[rows0 + nfull * 128:rows0 + rowsn, :],
                                      hb_t[:rem, nfull, :])

            nc.gpsimd.collective_compute(
                "AllGather", mybir.AluOpType.bypass, replica_groups=rg,
                ins=[ag_in.opt()], outs=[h0_full.opt()])

            # ---- layer-1 window loop -----------------------------------
            for w in range(NWIN):
                span = span_of(w)
                cw0 = co1[w][0]
                cwn = co1[w][NCHUNK - 1] + R1[w][NCHUNK - 1] - cw0

                idx_w = iop.tile([128, cwn * 8], I16, tag="idxw")
                nc.sync.dma_start(idx_w[:], idx1_d[:, cw0 * 8:(cw0 + cwn) * 8])
                segv_w = iop.tile([128, 2, cwn], F32, tag="segvw")
                nc.sync.dma_start(segv_w[:], segv1_d[:, :, cw0:cw0 + cwn])
                hTw = iop.tile([H, WIN], F32, tag="hTw")
                nc.sync.dma_start(hTw[:, :span],
                                  hT_dram[:, w * WIN:w * WIN + span])

                agg_ps = psp2.tile([H, WIN], F32, space="PSUM", tag="agg")
                nc.vector.memset(agg_ps[:, :], 0.0)

                mm = []
                for c in range(NCHUNK):
                    r = R1[w][c]
                    o = co1[w][c] - cw0
                    wg = gatp.tile([128, r, H], F32, tag=f"wg{c % 2}")
                    nc.gpsimd.dma_gather(
                        out_ap=wg[:, :, :],
                        in_ap=h0_full[c * CHUNK:min((c + 1) * CHUNK, N), :],
                        idxs_ap=idx_w[:, o * 8:(o + r) * 8],
                        num_idxs=r * 128, num_idxs_reg=r * 128, elem_size=H,
                        single_packet=False, queue_num=c % NSWQ)
                    for b0 in range(0, r, COLB):
                        nb = min(COLB, r - b0)
                        B = bbp.tile([128, COLB, SEGW], F32, tag="B")
                        seg_b = segv_w[:, 0, o + b0:o + b0 + nb].unsqueeze(2) \
                            .to_broadcast([128, nb, SEGW])
                        v_b = segv_w[:, 1, o + b0:o + b0 + nb].unsqueeze(2) \
                            .to_broadcast([128, nb, SEGW])
                        iota_b = iota32.unsqueeze(1).to_broadcast([128, nb, SEGW])
                        nc.vector.tensor_tensor(out=B[:, :nb, :], in0=seg_b, in1=iota_b,
                                                op=mybir.AluOpType.is_equal)
                        nc.vector.tensor_tensor(out=B[:, :nb, :], in0=B[:, :nb, :],
                                                in1=v_b, op=mybir.AluOpType.mult)
                        for t in range(nb):
                            j = b0 + t
                            f = F1[w][c][j]
                            sw = min(SEGW, span - f)
                            mm.append((wg, j, B, t, f, sw))
                for q, (wg, j, B, t, f, sw) in enumerate(mm):
                    nc.tensor.matmul(out=agg_ps[:, f:f + sw], lhsT=wg[:, j, :],
                                     rhs=B[:, t, :sw], start=False,
                                     stop=(q == len(mm) - 1), skip_group_check=True)

                aggT = iop.tile([H, WIN], F32, tag="aggT")
                nc.scalar.copy(aggT[:, :span], agg_ps[:, :span])

                z_ps = psp2.tile([H, WIN], F32, space="PSUM", tag="z")
                nc.tensor.matmul(out=z_ps[:, :span], lhsT=Wl1T, rhs=aggT[:, :span],
                                 start=True, stop=False)
                nc.tensor.matmul(out=z_ps[:, :span], lhsT=Wr1T,
                                 rhs=hTw[:, :span], start=False, stop=True)

                h1T_sb = iop.tile([H, WIN], F32, tag="h1T")
                nc.scalar.activation(h1T_sb[:, :span], z_ps[:, :span],
                                     mybir.ActivationFunctionType.Relu, bias=bl1)

                # s per 128-node block (column layout), t as row
                nblk = math.ceil(span / 128)
                s_ps = psp.tile([128, 4], F32, space="PSUM", tag="s")
                for b in range(nblk):
                    rows = min(128, span - b * 128)
                    nc.tensor.matmul(out=s_ps[:rows, b:b + 1],
                                     lhsT=h1T_sb[:, b * 128:b * 128 + rows],
                                     rhs=alpha, start=True, stop=True)
                if span == WIN:
                    nc.scalar.copy(s_sb[:, w * 4:w * 4 + nblk], s_ps[:, :nblk])
                else:
                    for b in range(nblk):
                        rows = min(128, span - b * 128)
                        nc.scalar.copy(s_sb[:rows, w * 4 + b:w * 4 + b + 1],
                                       s_ps[:rows, b:b + 1])
                t_ps = psp.tile([1, WIN], F32, space="PSUM", tag="s")
                nc.tensor.matmul(out=t_ps[:, :span], lhsT=beta,
                                 rhs=h1T_sb[:, :span], start=True, stop=True)
                t_sb = iop.tile([1, WIN], F32, tag="tsb")
                nc.scalar.activation(t_sb[:, :span], t_ps[:, :span],
                                     mybir.ActivationFunctionType.Identity,
                                     bias=c0)
                nc.sync.dma_start(t_dram[:, w * WIN:w * WIN + span],
                                  t_sb[:, :span])

            # ---- s AllGather -------------------------------------------
            nc.sync.dma_start(
                s_ag_in[:, :].rearrange("(t p) o -> p (t o)", p=128), s_sb[:, :])
            nc.gpsimd.collective_compute(
                "AllGather", mybir.AluOpType.bypass, replica_groups=rg,
                ins=[s_ag_in.opt()], outs=[s_full.opt()])

            # ---- layer-2 window loop (scalar messages) ------------------
            for w in range(NWIN):
                span = span_of(w)
                cw0 = co2[w][0]
                cwn = R2[w][0]

                idx2_w = iop.tile([128, cwn * 8], I16, tag="idx2w")
                nc.sync.dma_start(idx2_w[:], idx2_d[:, cw0 * 8:(cw0 + cwn) * 8])
                segv2_w = iop.tile([128, 3, cwn], F32, tag="segv2w")
                nc.sync.dma_start(segv2_w[:], segv2_d[:, :, cw0:cw0 + cwn])
                t_row = iop.tile([1, WIN], F32, tag="trow")
                nc.sync.dma_start(t_row[:, :span],
                                  t_dram[:, w * WIN:w * WIN + span])

                agg2_ps = psp2.tile([H, WIN], F32, space="PSUM", tag="agg")
                nc.vector.memset(agg2_ps[:, :], 0.0)

                # matmuls issued per column-block (slot frees early; no
                # deferred-slot deadlock however many gather blocks there are)
                q = 0
                for g0 in range(0, cwn, GB2):
                    gn = min(GB2, cwn - g0)
                    wg2 = gatp.tile([128, GB2, 64], F32, tag=f"wg2{(g0 // GB2) % 2}")
                    nc.gpsimd.dma_gather(
                        out_ap=wg2[:, :gn, :], in_ap=s_full[:, :],
                        idxs_ap=idx2_w[:, (g0) * 8:(g0 + gn) * 8],
                        num_idxs=gn * 128, num_idxs_reg=gn * 128, elem_size=64,
                        single_packet=False, queue_num=(g0 // GB2) % NSWQ)
                    for b0 in range(g0, g0 + gn, COLB):
                        nb = min(COLB, g0 + gn - b0)
                        # select mask: one-hot over 64 at off (invdeg is in B2)
                        SEL = bbp.tile([128, COLB, 64], F32, tag="SEL")
                        off_b = segv2_w[:, 2, b0:b0 + nb].unsqueeze(2) \
                            .to_broadcast([128, nb, 64])
                        iota_b = iota64.unsqueeze(1).to_broadcast([128, nb, 64])
                        nc.vector.tensor_tensor(out=SEL[:, :nb, :], in0=off_b,
                                                in1=iota_b,
                                                op=mybir.AluOpType.is_equal)
                        nc.vector.tensor_tensor(
                            out=wg2[:, b0 - g0:b0 - g0 + nb, :],
                            in0=wg2[:, b0 - g0:b0 - g0 + nb, :],
                            in1=SEL[:, :nb, :], op=mybir.AluOpType.mult)
                        B = bbp.tile([128, COLB, SEGW], F32, tag="B2")
                        seg_b = segv2_w[:, 0, b0:b0 + nb].unsqueeze(2) \
                            .to_broadcast([128, nb, SEGW])
                        v_b = segv2_w[:, 1, b0:b0 + nb].unsqueeze(2) \
                            .to_broadcast([128, nb, SEGW])
                        iotas_b = iota32.unsqueeze(1).to_broadcast([128, nb, SEGW])
                        nc.vector.tensor_tensor(out=B[:, :nb, :], in0=seg_b,
                                                in1=iotas_b,
                                                op=mybir.AluOpType.is_equal)
                        nc.vector.tensor_tensor(out=B[:, :nb, :], in0=B[:, :nb, :],
                                                in1=v_b, op=mybir.AluOpType.mult)
                        for t in range(nb):
                            j = b0 + t
                            f = F2[w][0][j]
                            sw = min(SEGW, span - f)
                            q += 1
                            nc.tensor.matmul(
                                out=agg2_ps[:, f:f + sw], lhsT=wg2[:, j - g0, :],
                                rhs=B[:, t, :sw], start=False, stop=(q == cwn),
                                skip_group_check=True)

                agg2T = iop.tile([H, WIN], F32, tag="aggT")
                nc.scalar.copy(agg2T[:, :span], agg2_ps[:, :span])
                o_ps = psp.tile([1, WIN], F32, space="PSUM", tag="s")
                nc.tensor.matmul(out=o_ps[:, :span], lhsT=ones,
                                 rhs=agg2T[:, :span], start=True, stop=True)
                out_sb = iop.tile([1, WIN], F32, tag="outsb")
                nc.vector.tensor_tensor(out=out_sb[:, :span], in0=o_ps[:, :span],
                                        in1=t_row[:, :span],
                                        op=mybir.AluOpType.add)
                nc.sync.dma_start(out_d[:, w * WIN:w * WIN + span],
                                  out_sb[:, :span])

    nc.compile()
    return nc


def kernel(x, edge_index, batch, fc_W, fc_b, Wl1, bl1, Wr1, Wl2, bl2, Wr2,
           head_W, head_b, cfg=None):
    cfg = cfg or Cfg(n_nodes=x.shape[0], in_dim=x.shape[1], hid=fc_W.shape[0])
    x = np.asarray(x, dtype=np.float32)
    src = np.asarray(edge_index[0], dtype=np.int64)
    dst = np.asarray(edge_index[1], dtype=np.int64)
    fc_W, fc_b, Wl1, bl1, Wr1, Wl2, bl2, Wr2, head_W, head_b = [
        np.asarray(a, dtype=np.float32)
        for a in (fc_W, fc_b, Wl1, bl1, Wr1, Wl2, bl2, Wr2, head_W, head_b)]

    t0 = time.time()
    p1, p2 = plan(cfg, src, dst)
    timings["plan_s"] = time.time() - t0
    wt, wcols = pack_weights(cfg, fc_W, fc_b, Wl1, bl1, Wr1,
                             Wl2, bl2, Wr2, head_W, head_b)

    t0 = time.time()
    ncb = build(cfg, p1, p2, wcols, wt.shape[1])
    timings["build_s"] = time.time() - t0

    in_maps = []
    for k in range(cfg.NC):
        in_maps.append({
            "wt": wt, "idx1": p1["idx"][k], "segv1": p1["segv"][k],
            "idx2": p2["idx"][k], "segv2": p2["segv"][k],
            "x_my": x[k * cfg.NPC:(k + 1) * cfg.NPC],
        })
    t0 = time.time()
    res = run_bass_kernel_spmd(ncb, in_maps, core_ids=list(range(cfg.NC)))
    timings["run_s"] = time.time() - t0
    timings["hw_ns"] = res.exec_time_ns
    out = np.concatenate([r["out"][0] for r in res.results], axis=0)
    return out.reshape(cfg.N, 1).astype(np.float32)


# revision 26
# speedup vs baseline: 1.9644x; 1.0112x over previous
"""Trainium2 Bass kernel for CustomGNN (fc+relu -> SAGEConv+relu -> SAGEConv -> head).

Single-NEFF SPMD design (8 NeuronCores, one program, per-core data):
  - nodes sharded in 8 contiguous chunks of 12500 (dst-partitioned edges).
  - fc stage sharded: each core computes h0 for its own nodes, then an
    AllGather builds the full [100000, 64] h0 table in every core's HBM.
  - SAGE layer 1: per 512-dst window, edges (sorted by dst, packed into
    128-slot columns) gather h0[src] rows (256B) with dma_gather from 4 src
    chunks (int16 index range); segment-sum via PE matmuls against one-hot
    inv-deg matrices built on DVE (is_equal vs iota over 32-wide sliding seg
    windows); then zT = Wl1 @ aggT + Wr1 @ h0T + bl1, relu -> h1T.
  - SAGE layer 2 + head collapsed to scalars (linearity of the head):
      out[v] = invdeg_v * sum_{u->v} s[u] + t[v]
      s[u] = h1[u] . (head_W @ Wl2),  t[v] = h1[v] . (head_W @ Wr2) + c0
    s is AllGathered (50KB/rank, padded shard stride 12544), then a second
    window pass gathers 64-value blocks of the dense s table (idx = padded
    row >> 6, fits int16 with no chunking), selects the scalar with a DVE
    one-hot over 64, and folds the 64-wide reduction into the PE
    segment-sum matmuls (final ones-vector partition reduce per window).

All index-derived structure (column quotas R, seg-window starts F) is computed
on the host from edge_index and baked into the program; quotas are max-merged
across the 8 cores so the SPMD program is identical on every core.
"""
import math
import time
import numpy as np

from concourse import bacc, mybir
from concourse.tile import TileContext
from concourse.bass_utils import run_bass_kernel_spmd

timings = {}

NSWQ = 2

F32 = mybir.dt.float32
I16 = mybir.dt.int16


class Cfg:
    def __init__(self, n_nodes=100000, in_dim=128, hid=64, ncores=8,
                 win=512, nchunk=4, segw=32, colb=16, xb=4, gb2=48):
        assert n_nodes % ncores == 0
        self.N = n_nodes
        self.D = in_dim
        self.H = hid
        self.NC = ncores
        self.NPC = n_nodes // ncores
        self.WIN = win
        self.NWIN = math.ceil(self.NPC / win)
        self.NCHUNK = nchunk
        self.CHUNK = math.ceil(n_nodes / nchunk)
        assert self.CHUNK <= 32767
        self.SEGW = segw
        self.COLB = colb
        self.XB = xb
        self.GB2 = gb2                        # layer-2 gather block (columns)
        self.NBLK = math.ceil(self.NPC / 128)
        self.SPAD = self.NBLK * 128           # padded shard stride (12544)
        assert self.SPAD % 64 == 0
        self.NB2 = self.SPAD * ncores // 64   # s AG output rows ([NB2, 64])
        self.NROW2 = self.SPAD * ncores       # expanded s table rows
        self.NCHUNK2 = nchunk
        self.CHUNK2 = math.ceil(self.NROW2 / self.NCHUNK2)
        assert self.CHUNK2 <= 32767


def _schedule_F(R, span, segw):
    if R <= 1:
        return [0] * max(R, 1)
    top = max(span - segw, 0)
    return [min((j * top) // (R - 1), top) for j in range(R)]


def _schedule_F_adaptive(segs_by_core, r, span, segw):
    """Place col windows at worst-case rank positions across cores: col j
    targets rank ~j*128; window start bounded above by every core's seg at
    that rank (no lagging edge) and nudged up to cover the window tail."""
    top = max(span - segw, 0)
    nmax = max((len(s) for s in segs_by_core), default=0)
    if r <= 1 or nmax == 0:
        return [0] * max(r, 1)
    F = []
    prev = 0
    for j in range(r):
        m = (j * (nmax - 1)) // (r - 1)
        hi = top
        lo = 0
        for s in segs_by_core:
            n = len(s)
            if n:
                hi = min(hi, int(s[min(m, n - 1)]))
                lo = max(lo, int(s[min(m + 127, n - 1)]) - (segw - 1))
        f = max(prev, min(hi, top))
        f = max(f, min(lo, top))
        F.append(f)
        prev = f
    return F


def _pack_group(segs, R, F, segw):
    """Greedy-pack sorted segs into R cols of <=128 edges, col j window
    [F[j], F[j]+segw). Returns list of (start, count) per col, or None."""
    n = len(segs)
    ptr = 0
    out = []
    for j in range(R):
        if ptr < n and segs[ptr] < F[j]:
            return None
        hi = np.searchsorted(segs, F[j] + segw, side="left")
        take = min(128, hi - ptr)
        out.append((ptr, take))
        ptr += take
    if ptr != n:
        return None
    return out


def _plan_pass(cfg, dst, seg_all_src, gid_base, nchunk, chunk, tabidx, extra):
    """Generic planning for one gather+segment-sum pass.

    dst: global dst per edge; chunk: chunk id per edge (grouping + gather
    table split); tabidx: per-edge gather index (relative to chunk base
    handled by caller); extra: dict name -> per-edge f32 array baked beside
    (seg, invdeg) into the segv tensor.
    Returns structure dict + per-core idx (int16, wrapped) and segv arrays.
    """
    N, NC, NPC, WIN, NWIN, SEGW = cfg.N, cfg.NC, cfg.NPC, cfg.WIN, cfg.NWIN, cfg.SEGW

    deg = np.bincount(dst, minlength=N)
    inv = (1.0 / np.maximum(deg, 1)).astype(np.float32)

    core = dst // NPC
    local = dst - core * NPC
    win = local // WIN
    seg = (local - win * WIN).astype(np.int32)

    gid = ((core * NWIN + win) * nchunk + chunk).astype(np.int64)
    order = np.lexsort((seg, gid))
    gids = gid[order]
    segs_all = seg[order]
    tab_all = tabidx[order]
    invdst_all = inv[dst[order]].astype(np.float32)
    extra_all = {k: v[order] for k, v in extra.items()}

    ngroups = NC * NWIN * nchunk
    counts = np.bincount(gids, minlength=ngroups)
    starts = np.zeros(ngroups + 1, dtype=np.int64)
    np.cumsum(counts, out=starts[1:])

    def span_of(w):
        return min(WIN, NPC - w * WIN)

    R = [[0] * nchunk for _ in range(NWIN)]
    F = [[None] * nchunk for _ in range(NWIN)]
    packs = {}
    for w in range(NWIN):
        span = span_of(w)
        for c in range(nchunk):
            r = 1
            segs_by_core = []
            for k in range(NC):
                g = (k * NWIN + w) * nchunk + c
                segs_by_core.append(segs_all[starts[g]:starts[g + 1]])
                r = max(r, math.ceil(max(counts[g], 1) / 128))
            while True:
                done = False
                for f in (_schedule_F_adaptive(segs_by_core, r, span, SEGW),
                          _schedule_F(r, span, SEGW)):
                    ok = True
                    for k in range(NC):
                        p = _pack_group(segs_by_core[k], r, f, SEGW)
                        if p is None:
                            ok = False
                            break
                        packs[(k, w, c)] = p
                    if ok:
                        done = True
                        break
                if done:
                    break
                r += 1
            R[w][c] = r
            F[w][c] = f

    col_off = [[0] * nchunk for _ in range(NWIN)]
    tot = 0
    for w in range(NWIN):
        for c in range(nchunk):
            col_off[w][c] = tot
            tot += R[w][c]
    totcols = tot

    nex = 2 + len(extra)
    exnames = sorted(extra.keys())
    idx_wrapped = []
    segv = []
    for k in range(NC):
        idxk = np.zeros((totcols, 128), dtype=np.int16)
        vals = np.zeros((nex, totcols, 128), dtype=np.float32)
        for w in range(NWIN):
            for c in range(nchunk):
                g = (k * NWIN + w) * nchunk + c
                s0 = starts[g]
                f = F[w][c]
                for j, (ptr, take) in enumerate(packs[(k, w, c)]):
                    if take <= 0:
                        continue
                    col = col_off[w][c] + j
                    sl = slice(s0 + ptr, s0 + ptr + take)
                    idxk[col, :take] = tab_all[sl].astype(np.int16)
                    vals[0, col, :take] = (segs_all[sl] - f[j]).astype(np.float32)
                    vals[1, col, :take] = invdst_all[sl]
                    for ei, en in enumerate(exnames):
                        vals[2 + ei, col, :take] = extra_all[en][sl]
        iw = idxk.reshape(totcols, 8, 16).transpose(2, 0, 1).reshape(16, totcols * 8)
        idx_wrapped.append(np.tile(iw, (8, 1)))
        segv.append(np.ascontiguousarray(vals.transpose(2, 0, 1)))  # [128, nex, totcols]

    return dict(R=R, F=F, col_off=col_off, totcols=totcols, nex=nex,
                idx=idx_wrapped, segv=segv, span_of=span_of)


def plan(cfg, src, dst):
    """Host planning for both gather passes."""
    chunk1 = src // cfg.CHUNK
    tab1 = (src - chunk1 * cfg.CHUNK).astype(np.int64)
    p1 = _plan_pass(cfg, dst, None, 0, cfg.NCHUNK, chunk1, tab1, {})

    # layer 2: gather 256B rows of the expanded s table (padded shard stride)
    kc = src // cfg.NPC
    padrow = kc * cfg.SPAD + (src - kc * cfg.NPC)
    chunk2 = padrow // cfg.CHUNK2
    tab2 = (padrow - chunk2 * cfg.CHUNK2).astype(np.int64)
    p2 = _plan_pass(cfg, dst, None, 0, cfg.NCHUNK2, chunk2, tab2, {})
    return p1, p2


def pack_weights(cfg, fc_W, fc_b, Wl1, bl1, Wr1, Wl2, bl2, Wr2, head_W, head_b):
    """One [128, WCOLS] f32 tensor with all constants."""
    D, H, SEGW = cfg.D, cfg.H, cfg.SEGW
    alpha = (head_W @ Wl2)[0]            # [H]
    beta = (head_W @ Wr2)[0]             # [H]
    c0 = float(head_b[0] + head_W[0] @ bl2)

    cols = {}
    seg = []

    def add(name, arr):
        arr = np.asarray(arr, np.float32)
        assert arr.ndim == 2 and arr.shape[0] <= 128
        cols[name] = sum(a.shape[1] for a in seg)
        pad = np.zeros((128, arr.shape[1]), np.float32)
        pad[:arr.shape[0]] = arr
        seg.append(pad)

    add("ident", np.eye(D, dtype=np.float32))
    add("iota32", np.tile(np.arange(SEGW, dtype=np.float32), (128, 1)))
    add("iota64", np.tile(np.arange(64, dtype=np.float32), (128, 1)))
    add("fcWT", fc_W.T)                  # [D, H]
    add("Wl1T", Wl1.T)                   # [H, H]
    add("Wr1T", Wr1.T)
    add("alpha", alpha[:, None])         # [H, 1]
    add("beta", beta[:, None])
    add("bl1", bl1[:, None])
    add("fcb", fc_b[:, None])
    add("ones", np.ones((H, 1), np.float32))
    add("c0", np.full((1, 1), c0, np.float32))
    wt = np.concatenate(seg, axis=1)
    return wt, cols


def build(cfg, p1, p2, wcols, wcols_n, ablate=()):
    N, D, H, NPC, WIN, NWIN = cfg.N, cfg.D, cfg.H, cfg.NPC, cfg.WIN, cfg.NWIN
    NCHUNK, CHUNK, SEGW, COLB, XB = cfg.NCHUNK, cfg.CHUNK, cfg.SEGW, cfg.COLB, cfg.XB
    NBLK, SPAD, NB2, GB2 = cfg.NBLK, cfg.SPAD, cfg.NB2, cfg.GB2
    R1, F1, co1, tc1 = p1["R"], p1["F"], p1["col_off"], p1["totcols"]
    R2, F2, co2, tc2 = p2["R"], p2["F"], p2["col_off"], p2["totcols"]
    span_of = p1["span_of"]

    nc = bacc.Bacc(None, target_bir_lowering=False, num_swdge_queues=NSWQ,
                   num_devices=cfg.NC)
    wt_d = nc.dram_tensor("wt", [128, wcols_n], F32, kind="ExternalInput")
    x_d = nc.dram_tensor("x_my", [NPC, D], F32, kind="ExternalInput")
    idx1_d = nc.dram_tensor("idx1", [128, tc1 * 8], I16, kind="ExternalInput")
    segv1_d = nc.dram_tensor("segv1", [128, 2, tc1], F32, kind="ExternalInput")
    idx2_d = nc.dram_tensor("idx2", [128, tc2 * 8], I16, kind="ExternalInput")
    segv2_d = nc.dram_tensor("segv2", [128, 2, tc2], F32, kind="ExternalInput")
    out_d = nc.dram_tensor("out", [1, NPC], F32, kind="ExternalOutput")

    rg = [list(range(cfg.NC))]

    with TileContext(nc) as tc:
        with (
            tc.tile_pool(name="dram", bufs=1, space="DRAM") as dramp,
            tc.tile_pool(name="const", bufs=1) as constp,
            tc.tile_pool(name="resid", bufs=1) as resid,
            tc.tile_pool(name="io", bufs=3) as iop,
            tc.tile_pool(name="gat", bufs=2) as gatp,
            tc.tile_pool(name="bb", bufs=3) as bbp,
            tc.tile_pool(name="ps", bufs=4, space="PSUM") as psp,
            tc.tile_pool(name="ps2", bufs=2, space="PSUM") as psp2,
        ):
            shared = "Local" if "nocc" in ablate else "Shared"
            ag_in = dramp.tile([NPC, H], F32)
            h0_full = dramp.tile([N, H], F32, addr_space=shared)
            hT_dram = dramp.tile([H, NPC], F32)
            t_dram = dramp.tile([1, NPC], F32)
            s_ag_in = dramp.tile([SPAD, 1], F32)
            s_full = dramp.tile([NB2, 64], F32, addr_space=shared)
            s_big = dramp.tile([cfg.NROW2, 64], F32)

            wt = constp.tile([128, wcols_n], F32)
            nc.sync.dma_start(wt[:], wt_d[:, :])
            ident = wt[:, wcols["ident"]:wcols["ident"] + D]
            iota32 = wt[:, wcols["iota32"]:wcols["iota32"] + SEGW]
            iota64 = wt[:, wcols["iota64"]:wcols["iota64"] + 64]
            fcWT = wt[:D, wcols["fcWT"]:wcols["fcWT"] + H]
            Wl1T = wt[:H, wcols["Wl1T"]:wcols["Wl1T"] + H]
            Wr1T = wt[:H, wcols["Wr1T"]:wcols["Wr1T"] + H]
            alpha = wt[:H, wcols["alpha"]:wcols["alpha"] + 1]
            beta = wt[:H, wcols["beta"]:wcols["beta"] + 1]
            bl1 = wt[:H, wcols["bl1"]:wcols["bl1"] + 1]
            fcb = wt[:H, wcols["fcb"]:wcols["fcb"] + 1]
            ones = wt[:H, wcols["ones"]:wcols["ones"] + 1]
            c0 = wt[0:1, wcols["c0"]:wcols["c0"] + 1]

            # s values, block-column layout: [p, b] = s[node b*128+p]
            s_sb = resid.tile([128, NBLK], F32)
            nc.vector.memset(s_sb[:, :], 0.0)

            # ---- fc stage over my shard: h0 rows -> ag_in, h0T -> hT_dram --
            nt = math.ceil(NPC / 128)
            for i0 in range(0, nt, XB):
                nb = min(XB, nt - i0)
                rows0 = i0 * 128
                rowsn = min(NPC - rows0, nb * 128)
                xb_t = iop.tile([128, XB, D], F32, tag="xb")
                nfull = rowsn // 128
                if nfull:
                    nc.sync.dma_start(
                        xb_t[:, :nfull, :],
                        x_d[rows0:rows0 + nfull * 128, :].rearrange(
                            "(t p) d -> p t d", p=128))
                rem = rowsn - nfull * 128
                if rem:
                    nc.sync.dma_start(xb_t[:rem, nfull, :],
                                      x_d[rows0 + nfull * 128:rows0 + rowsn, :])
                hb_t = iop.tile([128, XB, H], F32, tag="hb")
                hTb_t = iop.tile([H, XB, 128], F32, tag="hTb")
                for t in range(nb):
                    rows = min(128, NPC - rows0 - t * 128)
                    xT_ps = psp.tile([128, D], F32, space="PSUM", tag="s")
                    nc.tensor.transpose(xT_ps[:, :rows], xb_t[:rows, t, :],
                                        ident[:rows, :rows])
                    xT = iop.tile([128, D], F32, tag="xT_sb")
                    nc.vector.tensor_copy(xT[:], xT_ps[:])
                    hT_ps = psp.tile([H, 128], F32, space="PSUM", tag="s")
                    nc.tensor.matmul(out=hT_ps[:, :rows], lhsT=fcWT, rhs=xT[:, :rows],
                                     start=True, stop=True)
                    nc.scalar.activation(hTb_t[:, t, :rows], hT_ps[:, :rows],
                                         mybir.ActivationFunctionType.Relu, bias=fcb)
                    h_ps = psp.tile([128, H], F32, space="PSUM", tag="s")
                    nc.tensor.transpose(h_ps[:rows, :], hTb_t[:, t, :rows],
                                        ident[:H, :H])
                    nc.vector.tensor_copy(hb_t[:rows, t, :], h_ps[:rows, :])
                nc.sync.dma_start(hT_dram[:, rows0:rows0 + rowsn],
                                  hTb_t[:, :nb, :].rearrange(
                                      "h t p -> h (t p)")[:, :rowsn])
                if nfull:
                    nc.sync.dma_start(
                        ag_in[rows0:rows0 + nfull * 128, :].rearrange(
                            "(t p) d -> p t d", p=128),
                        hb_t[:, :nfull, :])
                if rem:
                    nc.sync.dma_start(ag_in[rows0 + nfull * 128:rows0 + rowsn, :],
                                      hb_t[:rem, nfull, :])

            if "nocc" in ablate:
                for kk in range(cfg.NC):
                    nc.sync.dma_start(h0_full[kk * NPC:(kk + 1) * NPC, :],
                                      ag_in[:, :])
            else:
                nc.gpsimd.collective_compute(
                    "AllGather", mybir.AluOpType.bypass, replica_groups=rg,
                    ins=[ag_in.opt()], outs=[h0_full.opt()])

            # ---- layer-1 window loop -----------------------------------
            for w in range(NWIN):
                span = span_of(w)
                cw0 = co1[w][0]
                cwn = co1[w][NCHUNK - 1] + R1[w][NCHUNK - 1] - cw0

                idx_w = iop.tile([128, cwn * 8], I16, tag="idxw")
                nc.sync.dma_start(idx_w[:], idx1_d[:, cw0 * 8:(cw0 + cwn) * 8])
                segv_w = iop.tile([128, 2, cwn], F32, tag="segvw")
                nc.sync.dma_start(segv_w[:], segv1_d[:, :, cw0:cw0 + cwn])
                hTw = iop.tile([H, WIN], F32, tag="hTw")
                nc.sync.dma_start(hTw[:, :span],
                                  hT_dram[:, w * WIN:w * WIN + span])

                agg_ps = psp2.tile([H, WIN], F32, space="PSUM", tag="agg")
                nc.vector.memset(agg_ps[:, :], 0.0)

                mm = []
                for c in range(NCHUNK):
                    if "nog1" in ablate:
                        break
                    r = R1[w][c]
                    o = co1[w][c] - cw0
                    wg = gatp.tile([128, r, H], F32, tag=f"wg{c % 2}")
                    nc.gpsimd.dma_gather(
                        out_ap=wg[:, :, :],
                        in_ap=h0_full[c * CHUNK:min((c + 1) * CHUNK, N), :],
                        idxs_ap=idx_w[:, o * 8:(o + r) * 8],
                        num_idxs=r * 128, num_idxs_reg=r * 128, elem_size=H,
                        single_packet=False, queue_num=c % NSWQ)
                    for b0 in range(0, r, COLB):
                        nb = min(COLB, r - b0)
                        B = bbp.tile([128, COLB, SEGW], F32, tag="B")
                        seg_b = segv_w[:, 0, o + b0:o + b0 + nb].unsqueeze(2) \
                            .to_broadcast([128, nb, SEGW])
                        v_b = segv_w[:, 1, o + b0:o + b0 + nb].unsqueeze(2) \
                            .to_broadcast([128, nb, SEGW])
                        iota_b = iota32.unsqueeze(1).to_broadcast([128, nb, SEGW])
                        nc.vector.tensor_tensor(out=B[:, :nb, :], in0=seg_b, in1=iota_b,
                                                op=mybir.AluOpType.is_equal)
                        nc.vector.tensor_tensor(out=B[:, :nb, :], in0=B[:, :nb, :],
                                                in1=v_b, op=mybir.AluOpType.mult)
                        for t in range(nb):
                            j = b0 + t
                            f = F1[w][c][j]
                            sw = min(SEGW, span - f)
                            mm.append((wg, j, B, t, f, sw))
                for q, (wg, j, B, t, f, sw) in enumerate(mm):
                    nc.tensor.matmul(out=agg_ps[:, f:f + sw], lhsT=wg[:, j, :],
                                     rhs=B[:, t, :sw], start=False,
                                     stop=(q == len(mm) - 1), skip_group_check=True)

                aggT = iop.tile([H, WIN], F32, tag="aggT")
                nc.scalar.copy(aggT[:, :span], agg_ps[:, :span])

                z_ps = psp2.tile([H, WIN], F32, space="PSUM", tag="z")
                nc.tensor.matmul(out=z_ps[:, :span], lhsT=Wl1T, rhs=aggT[:, :span],
                                 start=True, stop=False)
                nc.tensor.matmul(out=z_ps[:, :span], lhsT=Wr1T,
                                 rhs=hTw[:, :span], start=False, stop=True)

                h1T_sb = iop.tile([H, WIN], F32, tag="h1T")
                nc.scalar.activation(h1T_sb[:, :span], z_ps[:, :span],
                                     mybir.ActivationFunctionType.Relu, bias=bl1)

                # s per 128-node block (column layout), t as row
                nblk = math.ceil(span / 128)
                s_ps = psp.tile([128, 4], F32, space="PSUM", tag="s")
                for b in range(nblk):
                    rows = min(128, span - b * 128)
                    nc.tensor.matmul(out=s_ps[:rows, b:b + 1],
                                     lhsT=h1T_sb[:, b * 128:b * 128 + rows],
                                     rhs=alpha, start=True, stop=True)
                if span == WIN:
                    nc.scalar.copy(s_sb[:, w * 4:w * 4 + nblk], s_ps[:, :nblk])
                else:
                    for b in range(nblk):
                        rows = min(128, span - b * 128)
                        nc.scalar.copy(s_sb[:rows, w * 4 + b:w * 4 + b + 1],
                                       s_ps[:rows, b:b + 1])
                t_ps = psp.tile([1, WIN], F32, space="PSUM", tag="s")
                nc.tensor.matmul(out=t_ps[:, :span], lhsT=beta,
                                 rhs=h1T_sb[:, :span], start=True, stop=True)
                t_sb = iop.tile([1, WIN], F32, tag="tsb")
                nc.scalar.activation(t_sb[:, :span], t_ps[:, :span],
                                     mybir.ActivationFunctionType.Identity,
                                     bias=c0)
                nc.sync.dma_start(t_dram[:, w * WIN:w * WIN + span],
                                  t_sb[:, :span])

            # ---- s AllGather -------------------------------------------
            nc.sync.dma_start(
                s_ag_in[:, :].rearrange("(t p) o -> p (t o)", p=128), s_sb[:, :])
            if "nocc" in ablate:
                nr = SPAD // 64
                for kk in range(cfg.NC):
                    nc.sync.dma_start(
                        s_full[kk * nr:(kk + 1) * nr, :],
                        s_ag_in[:, :].rearrange("(r c) o -> r (c o)", c=64))
            else:
                nc.gpsimd.collective_compute(
                    "AllGather", mybir.AluOpType.bypass, replica_groups=rg,
                    ins=[s_ag_in.opt()], outs=[s_full.opt()])
            # expand: s_big[u, 0] = s_linear[u] (4 strided DMAs, 400KB total;
            # split keeps each transfer's dims under the 16-bit ISA field)
            nr4 = NB2 // cfg.NCHUNK2
            for c in range(cfg.NCHUNK2):
                nc.sync.dma_start(
                    s_big[c * nr4 * 64:(c + 1) * nr4 * 64, 0:1],
                    s_full[c * nr4:(c + 1) * nr4, :]
                    .rearrange("r c -> (r c)").unsqueeze(1))

            # ---- layer-2 window loop (scalar messages) ------------------
            if "nol2" in ablate:
                for w in range(NWIN):
                    span = span_of(w)
                    t_all = iop.tile([1, WIN], F32, tag="tall")
                    nc.sync.dma_start(t_all[:, :span],
                                      t_dram[:, w * WIN:w * WIN + span])
                    nc.sync.dma_start(out_d[:, w * WIN:w * WIN + span],
                                      t_all[:, :span])
            for w in range(NWIN if "nol2" not in ablate else 0):
                span = span_of(w)
                cw0 = co2[w][0]
                cwn = co2[w][cfg.NCHUNK2 - 1] + R2[w][cfg.NCHUNK2 - 1] - cw0

                idx2_w = iop.tile([128, cwn * 8], I16, tag="idx2w")
                nc.sync.dma_start(idx2_w[:], idx2_d[:, cw0 * 8:(cw0 + cwn) * 8])
                segv2_w = iop.tile([128, 2, cwn], F32, tag="segv2w")
                nc.sync.dma_start(segv2_w[:], segv2_d[:, :, cw0:cw0 + cwn])
                t_row = iop.tile([1, WIN], F32, tag="trow")
                nc.sync.dma_start(t_row[:, :span],
                                  t_dram[:, w * WIN:w * WIN + span])

                agg2_ps = psp2.tile([1, WIN], F32, space="PSUM", tag="agg")
                nc.vector.memset(agg2_ps[:, :], 0.0)

                mm = []
                for c in range(cfg.NCHUNK2):
                    if "nog2" in ablate:
                        break
                    r = R2[w][c]
                    o = co2[w][c] - cw0
                    wg2 = gatp.tile([128, r, 64], F32, tag=f"wg2{c % 2}")
                    nc.gpsimd.dma_gather(
                        out_ap=wg2[:, :, :],
                        in_ap=s_big[c * cfg.CHUNK2:
                                    min((c + 1) * cfg.CHUNK2, cfg.NROW2), :],
                        idxs_ap=idx2_w[:, o * 8:(o + r) * 8],
                        num_idxs=r * 128, num_idxs_reg=r * 128, elem_size=64,
                        single_packet=False, queue_num=c % NSWQ)
                    for b0 in range(0, r if "g2only" not in ablate else 0, COLB):
                        nb = min(COLB, r - b0)
                        B = bbp.tile([128, COLB, SEGW], F32, tag="B2")
                        seg_b = segv2_w[:, 0, o + b0:o + b0 + nb].unsqueeze(2) \
                            .to_broadcast([128, nb, SEGW])
                        v_b = segv2_w[:, 1, o + b0:o + b0 + nb].unsqueeze(2) \
                            .to_broadcast([128, nb, SEGW])
                        iotas_b = iota32.unsqueeze(1).to_broadcast([128, nb, SEGW])
                        nc.vector.tensor_tensor(out=B[:, :nb, :], in0=seg_b,
                                                in1=iotas_b,
                                                op=mybir.AluOpType.is_equal)
                        nc.vector.tensor_tensor(out=B[:, :nb, :], in0=B[:, :nb, :],
                                                in1=v_b, op=mybir.AluOpType.mult)
                        for t in range(nb):
                            j = b0 + t
                            f = F2[w][c][j]
                            sw = min(SEGW, span - f)
                            mm.append((wg2, j, B, t, f, sw))
                for q, (wg2, j, B, t, f, sw) in enumerate(mm):
                    nc.tensor.matmul(out=agg2_ps[:, f:f + sw],
                                     lhsT=wg2[:, j, 0:1], rhs=B[:, t, :sw],
                                     start=False, stop=(q == len(mm) - 1),
                                     skip_group_check=True)

                out_sb = iop.tile([1, WIN], F32, tag="outsb")
                nc.vector.tensor_tensor(out=out_sb[:, :span],
                                        in0=agg2_ps[:, :span],
                                        in1=t_row[:, :span],
                                        op=mybir.AluOpType.add)
                nc.sync.dma_start(out_d[:, w * WIN:w * WIN + span],
                                  out_sb[:, :span])

    nc.compile()
    return nc


def kernel(x, edge_index, batch, fc_W, fc_b, Wl1, bl1, Wr1, Wl2, bl2, Wr2,
           head_W, head_b, cfg=None):
    cfg = cfg or Cfg(n_nodes=x.shape[0], in_dim=x.shape[1], hid=fc_W.shape[0])
    x = np.asarray(x, dtype=np.float32)
    src = np.asarray(edge_index[0], dtype=np.int64)
    dst = np.asarray(edge_index[1], dtype=np.int64)
    fc_W, fc_b, Wl1, bl1, Wr1, Wl2, bl2, Wr2, head_W, head_b = [
        np.asarray(a, dtype=np.float32)
        for a in (fc_W, fc_b, Wl1, bl1, Wr1, Wl2, bl2, Wr2, head_W, head_b)]

    t0 = time.time()
    p1, p2 = plan(cfg, src, dst)
    timings["plan_s"] = time.time() - t0
    wt, wcols = pack_weights(cfg, fc_W, fc_b, Wl1, bl1, Wr1,
                             Wl2, bl2, Wr2, head_W, head_b)

    t0 = time.time()
    ncb = build(cfg, p1, p2, wcols, wt.shape[1])
    timings["build_s"] = time.time() - t0

    in_maps = []
    for k in range(cfg.NC):
        in_maps.append({
            "wt": wt, "idx1": p1["idx"][k], "segv1": p1["segv"][k],
            "idx2": p2["idx"][k], "segv2": p2["segv"][k],
            "x_my": x[k * cfg.NPC:(k + 1) * cfg.NPC],
        })
    t0 = time.time()
    res = run_bass_kernel_spmd(ncb, in_maps, core_ids=list(range(cfg.NC)))
    timings["run_s"] = time.time() - t0
    timings["hw_ns"] = res.exec_time_ns
    out = np.concatenate([r["out"][0] for r in res.results], axis=0)
    return out.reshape(cfg.N, 1).astype(np.float32)


# revision 31
# speedup vs baseline: 1.9703x; 1.0030x over previous
"""Trainium2 Bass kernel for CustomGNN (fc+relu -> SAGEConv+relu -> SAGEConv -> head).

Single-NEFF SPMD design (8 NeuronCores, one program, per-core data):
  - nodes sharded in 8 contiguous chunks of 12500 (dst-partitioned edges).
  - fc stage sharded: each core computes h0 for its own nodes, then an
    AllGather builds the full [100000, 64] h0 table in every core's HBM.
  - SAGE layer 1: per 512-dst window, edges (sorted by dst, packed into
    128-slot columns) gather h0[src] rows (256B) with dma_gather from 4 src
    chunks (int16 index range); segment-sum via PE matmuls against one-hot
    inv-deg matrices built on DVE (is_equal vs iota over 32-wide sliding seg
    windows); then zT = Wl1 @ aggT + Wr1 @ h0T + bl1, relu -> h1T.
  - SAGE layer 2 + head collapsed to scalars (linearity of the head):
      out[v] = invdeg_v * sum_{u->v} s[u] + t[v]
      s[u] = h1[u] . (head_W @ Wl2),  t[v] = h1[v] . (head_W @ Wr2) + c0
    s is AllGathered (50KB/rank, padded shard stride 12544), expanded into a
    [100352, 64] row-table (s at column 0 of each 256B row, one strided DMA
    per chunk), then a second window pass mirrors layer 1: dma_gather 256B
    rows from 4 int16 chunks and PE segment-sum with 1-wide lhsT (column 0),
    accumulating straight into a [1, 512] PSUM row added to t.

All index-derived structure (column quotas R, seg-window starts F) is computed
on the host from edge_index and baked into the program; quotas are max-merged
across the 8 cores so the SPMD program is identical on every core.
"""
import math
import time
import numpy as np

from concourse import bacc, mybir
from concourse.tile import TileContext
from concourse.bass_utils import run_bass_kernel_spmd

timings = {}

NSWQ = 2

F32 = mybir.dt.float32
I16 = mybir.dt.int16


class Cfg:
    def __init__(self, n_nodes=100000, in_dim=128, hid=64, ncores=8,
                 win=512, nchunk=4, segw=32, colb=32, xb=4, gb2=48, nswq=4):
        assert n_nodes % ncores == 0
        self.N = n_nodes
        self.D = in_dim
        self.H = hid
        self.NC = ncores
        self.NPC = n_nodes // ncores
        self.WIN = win
        self.NWIN = math.ceil(self.NPC / win)
        self.NCHUNK = nchunk
        self.CHUNK = math.ceil(n_nodes / nchunk)
        assert self.CHUNK <= 32767
        self.SEGW = segw
        self.COLB = colb
        self.XB = xb
        self.GB2 = gb2                        # layer-2 gather block (columns)
        self.NSWQ = nswq
        self.NBLK = math.ceil(self.NPC / 128)
        self.SPAD = self.NBLK * 128           # padded shard stride (12544)
        assert self.SPAD % 64 == 0
        self.NB2 = self.SPAD * ncores // 64   # s AG output rows ([NB2, 64])
        self.NROW2 = self.SPAD * ncores       # expanded s table rows
        self.NCHUNK2 = nchunk
        self.CHUNK2 = math.ceil(self.NROW2 / self.NCHUNK2)
        assert self.CHUNK2 <= 32767


def _schedule_F(R, span, segw):
    if R <= 1:
        return [0] * max(R, 1)
    top = max(span - segw, 0)
    return [min((j * top) // (R - 1), top) for j in range(R)]


def _schedule_F_adaptive(segs_by_core, r, span, segw):
    """Place col windows at worst-case rank positions across cores: col j
    targets rank ~j*128; window start bounded above by every core's seg at
    that rank (no lagging edge) and nudged up to cover the window tail."""
    top = max(span - segw, 0)
    nmax = max((len(s) for s in segs_by_core), default=0)
    if r <= 1 or nmax == 0:
        return [0] * max(r, 1)
    F = []
    prev = 0
    for j in range(r):
        m = (j * (nmax - 1)) // (r - 1)
        hi = top
        lo = 0
        for s in segs_by_core:
            n = len(s)
            if n:
                hi = min(hi, int(s[min(m, n - 1)]))
                lo = max(lo, int(s[min(m + 127, n - 1)]) - (segw - 1))
        f = max(prev, min(hi, top))
        f = max(f, min(lo, top))
        F.append(f)
        prev = f
    return F


def _pack_group(segs, R, F, segw):
    """Greedy-pack sorted segs into R cols of <=128 edges, col j window
    [F[j], F[j]+segw). Returns list of (start, count) per col, or None."""
    n = len(segs)
    ptr = 0
    out = []
    for j in range(R):
        if ptr < n and segs[ptr] < F[j]:
            return None
        hi = np.searchsorted(segs, F[j] + segw, side="left")
        take = min(128, hi - ptr)
        out.append((ptr, take))
        ptr += take
    if ptr != n:
        return None
    return out


def _plan_pass(cfg, dst, seg_all_src, gid_base, nchunk, chunk, tabidx, extra):
    """Generic planning for one gather+segment-sum pass.

    dst: global dst per edge; chunk: chunk id per edge (grouping + gather
    table split); tabidx: per-edge gather index (relative to chunk base
    handled by caller); extra: dict name -> per-edge f32 array baked beside
    (seg, invdeg) into the segv tensor.
    Returns structure dict + per-core idx (int16, wrapped) and segv arrays.
    """
    N, NC, NPC, WIN, NWIN, SEGW = cfg.N, cfg.NC, cfg.NPC, cfg.WIN, cfg.NWIN, cfg.SEGW

    deg = np.bincount(dst, minlength=N)
    inv = (1.0 / np.maximum(deg, 1)).astype(np.float32)

    core = dst // NPC
    local = dst - core * NPC
    win = local // WIN
    seg = (local - win * WIN).astype(np.int32)

    gid = ((core * NWIN + win) * nchunk + chunk).astype(np.int64)
    order = np.lexsort((seg, gid))
    gids = gid[order]
    segs_all = seg[order]
    tab_all = tabidx[order]
    invdst_all = inv[dst[order]].astype(np.float32)
    extra_all = {k: v[order] for k, v in extra.items()}

    ngroups = NC * NWIN * nchunk
    counts = np.bincount(gids, minlength=ngroups)
    starts = np.zeros(ngroups + 1, dtype=np.int64)
    np.cumsum(counts, out=starts[1:])

    def span_of(w):
        return min(WIN, NPC - w * WIN)

    R = [[0] * nchunk for _ in range(NWIN)]
    F = [[None] * nchunk for _ in range(NWIN)]
    packs = {}
    for w in range(NWIN):
        span = span_of(w)
        for c in range(nchunk):
            r = 1
            segs_by_core = []
            for k in range(NC):
                g = (k * NWIN + w) * nchunk + c
                segs_by_core.append(segs_all[starts[g]:starts[g + 1]])
                r = max(r, math.ceil(max(counts[g], 1) / 128))
            while True:
                done = False
                for f in (_schedule_F_adaptive(segs_by_core, r, span, SEGW),
                          _schedule_F(r, span, SEGW)):
                    ok = True
                    for k in range(NC):
                        p = _pack_group(segs_by_core[k], r, f, SEGW)
                        if p is None:
                            ok = False
                            break
                        packs[(k, w, c)] = p
                    if ok:
                        done = True
                        break
                if done:
                    break
                r += 1
            R[w][c] = r
            F[w][c] = f

    col_off = [[0] * nchunk for _ in range(NWIN)]
    tot = 0
    for w in range(NWIN):
        for c in range(nchunk):
            col_off[w][c] = tot
            tot += R[w][c]
    totcols = tot

    nex = 2 + len(extra)
    exnames = sorted(extra.keys())
    idx_wrapped = []
    segv = []
    for k in range(NC):
        idxk = np.zeros((totcols, 128), dtype=np.int16)
        vals = np.zeros((nex, totcols, 128), dtype=np.float32)
        for w in range(NWIN):
            for c in range(nchunk):
                g = (k * NWIN + w) * nchunk + c
                s0 = starts[g]
                f = F[w][c]
                for j, (ptr, take) in enumerate(packs[(k, w, c)]):
                    if take <= 0:
                        continue
                    col = col_off[w][c] + j
                    sl = slice(s0 + ptr, s0 + ptr + take)
                    idxk[col, :take] = tab_all[sl].astype(np.int16)
                    vals[0, col, :take] = (segs_all[sl] - f[j]).astype(np.float32)
                    vals[1, col, :take] = invdst_all[sl]
                    for ei, en in enumerate(exnames):
                        vals[2 + ei, col, :take] = extra_all[en][sl]
        iw = idxk.reshape(totcols, 8, 16).transpose(2, 0, 1).reshape(16, totcols * 8)
        idx_wrapped.append(np.tile(iw, (8, 1)))
        segv.append(np.ascontiguousarray(vals.transpose(2, 0, 1)))  # [128, nex, totcols]

    return dict(R=R, F=F, col_off=col_off, totcols=totcols, nex=nex,
                idx=idx_wrapped, segv=segv, span_of=span_of)


def plan(cfg, src, dst):
    """Host planning for both gather passes."""
    chunk1 = src // cfg.CHUNK
    tab1 = (src - chunk1 * cfg.CHUNK).astype(np.int64)
    p1 = _plan_pass(cfg, dst, None, 0, cfg.NCHUNK, chunk1, tab1, {})

    # layer 2: gather 256B rows of the expanded s table (padded shard stride)
    kc = src // cfg.NPC
    padrow = kc * cfg.SPAD + (src - kc * cfg.NPC)
    chunk2 = padrow // cfg.CHUNK2
    tab2 = (padrow - chunk2 * cfg.CHUNK2).astype(np.int64)
    p2 = _plan_pass(cfg, dst, None, 0, cfg.NCHUNK2, chunk2, tab2, {})
    return p1, p2


def pack_weights(cfg, fc_W, fc_b, Wl1, bl1, Wr1, Wl2, bl2, Wr2, head_W, head_b):
    """One [128, WCOLS] f32 tensor with all constants."""
    D, H, SEGW = cfg.D, cfg.H, cfg.SEGW
    alpha = (head_W @ Wl2)[0]            # [H]
    beta = (head_W @ Wr2)[0]             # [H]
    c0 = float(head_b[0] + head_W[0] @ bl2)

    cols = {}
    seg = []

    def add(name, arr):
        arr = np.asarray(arr, np.float32)
        assert arr.ndim == 2 and arr.shape[0] <= 128
        cols[name] = sum(a.shape[1] for a in seg)
        pad = np.zeros((128, arr.shape[1]), np.float32)
        pad[:arr.shape[0]] = arr
        seg.append(pad)

    add("ident", np.eye(D, dtype=np.float32))
    add("iota32", np.tile(np.arange(SEGW, dtype=np.float32), (128, 1)))
    add("iota64", np.tile(np.arange(64, dtype=np.float32), (128, 1)))
    add("fcWT", fc_W.T)                  # [D, H]
    add("Wl1T", Wl1.T)                   # [H, H]
    add("Wr1T", Wr1.T)
    add("alpha", alpha[:, None])         # [H, 1]
    add("beta", beta[:, None])
    add("bl1", bl1[:, None])
    add("fcb", fc_b[:, None])
    add("ones", np.ones((H, 1), np.float32))
    add("c0", np.full((1, 1), c0, np.float32))
    wt = np.concatenate(seg, axis=1)
    return wt, cols


def build(cfg, p1, p2, wcols, wcols_n, ablate=()):
    N, D, H, NPC, WIN, NWIN = cfg.N, cfg.D, cfg.H, cfg.NPC, cfg.WIN, cfg.NWIN
    NCHUNK, CHUNK, SEGW, COLB, XB = cfg.NCHUNK, cfg.CHUNK, cfg.SEGW, cfg.COLB, cfg.XB
    NBLK, SPAD, NB2, GB2 = cfg.NBLK, cfg.SPAD, cfg.NB2, cfg.GB2
    R1, F1, co1, tc1 = p1["R"], p1["F"], p1["col_off"], p1["totcols"]
    R2, F2, co2, tc2 = p2["R"], p2["F"], p2["col_off"], p2["totcols"]
    span_of = p1["span_of"]

    nc = bacc.Bacc(None, target_bir_lowering=False, num_swdge_queues=cfg.NSWQ,
                   num_devices=cfg.NC)
    wt_d = nc.dram_tensor("wt", [128, wcols_n], F32, kind="ExternalInput")
    x_d = nc.dram_tensor("x_my", [NPC, D], F32, kind="ExternalInput")
    idx1_d = nc.dram_tensor("idx1", [128, tc1 * 8], I16, kind="ExternalInput")
    segv1_d = nc.dram_tensor("segv1", [128, 2, tc1], F32, kind="ExternalInput")
    idx2_d = nc.dram_tensor("idx2", [128, tc2 * 8], I16, kind="ExternalInput")
    segv2_d = nc.dram_tensor("segv2", [128, 2, tc2], F32, kind="ExternalInput")
    out_d = nc.dram_tensor("out", [1, NPC], F32, kind="ExternalOutput")

    rg = [list(range(cfg.NC))]

    with TileContext(nc) as tc:
        with (
            tc.tile_pool(name="dram", bufs=1, space="DRAM") as dramp,
            tc.tile_pool(name="const", bufs=1) as constp,
            tc.tile_pool(name="resid", bufs=1) as resid,
            tc.tile_pool(name="io", bufs=3) as iop,
            tc.tile_pool(name="gat", bufs=2) as gatp,
            tc.tile_pool(name="bb", bufs=3) as bbp,
            tc.tile_pool(name="ps", bufs=4, space="PSUM") as psp,
            tc.tile_pool(name="ps2", bufs=2, space="PSUM") as psp2,
        ):
            shared = "Local" if "nocc" in ablate else "Shared"
            ag_in = dramp.tile([NPC, H], F32)
            h0_full = dramp.tile([N, H], F32, addr_space=shared)
            hT_dram = dramp.tile([H, NPC], F32)
            t_dram = dramp.tile([1, NPC], F32)
            s_ag_in = dramp.tile([SPAD, 1], F32)
            s_full = dramp.tile([NB2, 64], F32, addr_space=shared)
            s_big = dramp.tile([cfg.NROW2, 64], F32)

            wt = constp.tile([128, wcols_n], F32)
            nc.sync.dma_start(wt[:], wt_d[:, :])
            ident = wt[:, wcols["ident"]:wcols["ident"] + D]
            iota32 = wt[:, wcols["iota32"]:wcols["iota32"] + SEGW]
            iota64 = wt[:, wcols["iota64"]:wcols["iota64"] + 64]
            fcWT = wt[:D, wcols["fcWT"]:wcols["fcWT"] + H]
            Wl1T = wt[:H, wcols["Wl1T"]:wcols["Wl1T"] + H]
            Wr1T = wt[:H, wcols["Wr1T"]:wcols["Wr1T"] + H]
            alpha = wt[:H, wcols["alpha"]:wcols["alpha"] + 1]
            beta = wt[:H, wcols["beta"]:wcols["beta"] + 1]
            bl1 = wt[:H, wcols["bl1"]:wcols["bl1"] + 1]
            fcb = wt[:H, wcols["fcb"]:wcols["fcb"] + 1]
            ones = wt[:H, wcols["ones"]:wcols["ones"] + 1]
            c0 = wt[0:1, wcols["c0"]:wcols["c0"] + 1]

            # s values, block-column layout: [p, b] = s[node b*128+p]
            s_sb = resid.tile([128, NBLK], F32)
            nc.vector.memset(s_sb[:, :], 0.0)

            # ---- fc stage over my shard: h0 rows -> ag_in, h0T -> hT_dram --
            nt = math.ceil(NPC / 128)
            for i0 in range(0, nt, XB):
                nb = min(XB, nt - i0)
                rows0 = i0 * 128
                rowsn = min(NPC - rows0, nb * 128)
                xb_t = iop.tile([128, XB, D], F32, tag="xb")
                nfull = rowsn // 128
                if nfull:
                    nc.sync.dma_start(
                        xb_t[:, :nfull, :],
                        x_d[rows0:rows0 + nfull * 128, :].rearrange(
                            "(t p) d -> p t d", p=128))
                rem = rowsn - nfull * 128
                if rem:
                    nc.sync.dma_start(xb_t[:rem, nfull, :],
                                      x_d[rows0 + nfull * 128:rows0 + rowsn, :])
                hb_t = iop.tile([128, XB, H], F32, tag="hb")
                hTb_t = iop.tile([H, XB, 128], F32, tag="hTb")
                for t in range(nb):
                    rows = min(128, NPC - rows0 - t * 128)
                    xT_ps = psp.tile([128, D], F32, space="PSUM", tag="s")
                    nc.tensor.transpose(xT_ps[:, :rows], xb_t[:rows, t, :],
                                        ident[:rows, :rows])
                    xT = iop.tile([128, D], F32, tag="xT_sb")
                    nc.vector.tensor_copy(xT[:], xT_ps[:])
                    hT_ps = psp.tile([H, 128], F32, space="PSUM", tag="s")
                    nc.tensor.matmul(out=hT_ps[:, :rows], lhsT=fcWT, rhs=xT[:, :rows],
                                     start=True, stop=True)
                    nc.scalar.activation(hTb_t[:, t, :rows], hT_ps[:, :rows],
                                         mybir.ActivationFunctionType.Relu, bias=fcb)
                    h_ps = psp.tile([128, H], F32, space="PSUM", tag="s")
                    nc.tensor.transpose(h_ps[:rows, :], hTb_t[:, t, :rows],
                                        ident[:H, :H])
                    nc.vector.tensor_copy(hb_t[:rows, t, :], h_ps[:rows, :])
                nc.sync.dma_start(hT_dram[:, rows0:rows0 + rowsn],
                                  hTb_t[:, :nb, :].rearrange(
                                      "h t p -> h (t p)")[:, :rowsn])
                if nfull:
                    nc.sync.dma_start(
                        ag_in[rows0:rows0 + nfull * 128, :].rearrange(
                            "(t p) d -> p t d", p=128),
                        hb_t[:, :nfull, :])
                if rem:
                    nc.sync.dma_start(ag_in[rows0 + nfull * 128:rows0 + rowsn, :],
                                      hb_t[:rem, nfull, :])

            if "nocc" in ablate:
                for kk in range(cfg.NC):
                    nc.sync.dma_start(h0_full[kk * NPC:(kk + 1) * NPC, :],
                                      ag_in[:, :])
            else:
                nc.gpsimd.collective_compute(
                    "AllGather", mybir.AluOpType.bypass, replica_groups=rg,
                    ins=[ag_in.opt()], outs=[h0_full.opt()])

            # ---- layer-1 window loop -----------------------------------
            for w in range(NWIN):
                span = span_of(w)
                cw0 = co1[w][0]
                cwn = co1[w][NCHUNK - 1] + R1[w][NCHUNK - 1] - cw0

                idx_w = iop.tile([128, cwn * 8], I16, tag="idxw")
                nc.sync.dma_start(idx_w[:], idx1_d[:, cw0 * 8:(cw0 + cwn) * 8])
                segv_w = iop.tile([128, 2, cwn], F32, tag="segvw")
                nc.sync.dma_start(segv_w[:], segv1_d[:, :, cw0:cw0 + cwn])
                hTw = iop.tile([H, WIN], F32, tag="hTw")
                nc.sync.dma_start(hTw[:, :span],
                                  hT_dram[:, w * WIN:w * WIN + span])

                agg_ps = psp2.tile([H, WIN], F32, space="PSUM", tag="agg")
                nc.vector.memset(agg_ps[:, :], 0.0)

                mm = []
                for c in range(NCHUNK):
                    if "nog1" in ablate:
                        break
                    r = R1[w][c]
                    o = co1[w][c] - cw0
                    wg = gatp.tile([128, r, H], F32, tag=f"wg{c % 2}")
                    nc.gpsimd.dma_gather(
                        out_ap=wg[:, :, :],
                        in_ap=h0_full[c * CHUNK:min((c + 1) * CHUNK, N), :],
                        idxs_ap=idx_w[:, o * 8:(o + r) * 8],
                        num_idxs=r * 128, num_idxs_reg=r * 128, elem_size=H,
                        single_packet=False, queue_num=c % cfg.NSWQ)
                    for b0 in range(0, r, COLB):
                        nb = min(COLB, r - b0)
                        B = bbp.tile([128, COLB, SEGW], F32, tag="B")
                        seg_b = segv_w[:, 0, o + b0:o + b0 + nb].unsqueeze(2) \
                            .to_broadcast([128, nb, SEGW])
                        v_b = segv_w[:, 1, o + b0:o + b0 + nb].unsqueeze(2) \
                            .to_broadcast([128, nb, SEGW])
                        iota_b = iota32.unsqueeze(1).to_broadcast([128, nb, SEGW])
                        nc.vector.tensor_tensor(out=B[:, :nb, :], in0=seg_b, in1=iota_b,
                                                op=mybir.AluOpType.is_equal)
                        nc.vector.tensor_tensor(out=B[:, :nb, :], in0=B[:, :nb, :],
                                                in1=v_b, op=mybir.AluOpType.mult)
                        for t in range(nb):
                            j = b0 + t
                            f = F1[w][c][j]
                            sw = min(SEGW, span - f)
                            mm.append((wg, j, B, t, f, sw))
                for q, (wg, j, B, t, f, sw) in enumerate(mm):
                    nc.tensor.matmul(out=agg_ps[:, f:f + sw], lhsT=wg[:, j, :],
                                     rhs=B[:, t, :sw], start=False,
                                     stop=(q == len(mm) - 1), skip_group_check=True)

                aggT = iop.tile([H, WIN], F32, tag="aggT")
                nc.scalar.copy(aggT[:, :span], agg_ps[:, :span])

                z_ps = psp2.tile([H, WIN], F32, space="PSUM", tag="z")
                nc.tensor.matmul(out=z_ps[:, :span], lhsT=Wl1T, rhs=aggT[:, :span],
                                 start=True, stop=False)
                nc.tensor.matmul(out=z_ps[:, :span], lhsT=Wr1T,
                                 rhs=hTw[:, :span], start=False, stop=True)

                h1T_sb = iop.tile([H, WIN], F32, tag="h1T")
                nc.scalar.activation(h1T_sb[:, :span], z_ps[:, :span],
                                     mybir.ActivationFunctionType.Relu, bias=bl1)

                # s per 128-node block (column layout), t as row
                nblk = math.ceil(span / 128)
                s_ps = psp.tile([128, 4], F32, space="PSUM", tag="s")
                for b in range(nblk):
                    rows = min(128, span - b * 128)
                    nc.tensor.matmul(out=s_ps[:rows, b:b + 1],
                                     lhsT=h1T_sb[:, b * 128:b * 128 + rows],
                                     rhs=alpha, start=True, stop=True)
                if span == WIN:
                    nc.scalar.copy(s_sb[:, w * 4:w * 4 + nblk], s_ps[:, :nblk])
                else:
                    for b in range(nblk):
                        rows = min(128, span - b * 128)
                        nc.scalar.copy(s_sb[:rows, w * 4 + b:w * 4 + b + 1],
                                       s_ps[:rows, b:b + 1])
                t_ps = psp.tile([1, WIN], F32, space="PSUM", tag="s")
                nc.tensor.matmul(out=t_ps[:, :span], lhsT=beta,
                                 rhs=h1T_sb[:, :span], start=True, stop=True)
                t_sb = iop.tile([1, WIN], F32, tag="tsb")
                nc.scalar.activation(t_sb[:, :span], t_ps[:, :span],
                                     mybir.ActivationFunctionType.Identity,
                                     bias=c0)
                nc.sync.dma_start(t_dram[:, w * WIN:w * WIN + span],
                                  t_sb[:, :span])

            # ---- s AllGather -------------------------------------------
            nc.sync.dma_start(
                s_ag_in[:, :].rearrange("(t p) o -> p (t o)", p=128), s_sb[:, :])
            if "nocc" in ablate:
                nr = SPAD // 64
                for kk in range(cfg.NC):
                    nc.sync.dma_start(
                        s_full[kk * nr:(kk + 1) * nr, :],
                        s_ag_in[:, :].rearrange("(r c) o -> r (c o)", c=64))
            else:
                nc.gpsimd.collective_compute(
                    "AllGather", mybir.AluOpType.bypass, replica_groups=rg,
                    ins=[s_ag_in.opt()], outs=[s_full.opt()])
            # expand: s_big[u, 0] = s_linear[u] (4 strided DMAs, 400KB total;
            # split keeps each transfer's dims under the 16-bit ISA field)
            nr4 = NB2 // cfg.NCHUNK2
            for c in range(cfg.NCHUNK2):
                nc.sync.dma_start(
                    s_big[c * nr4 * 64:(c + 1) * nr4 * 64, 0:1],
                    s_full[c * nr4:(c + 1) * nr4, :]
                    .rearrange("r c -> (r c)").unsqueeze(1))

            # ---- layer-2 window loop (scalar messages) ------------------
            if "nol2" in ablate:
                for w in range(NWIN):
                    span = span_of(w)
                    cw0 = co2[w][0]
                    cwn = co2[w][cfg.NCHUNK2 - 1] + R2[w][cfg.NCHUNK2 - 1] - cw0
                    if "only_idx" in ablate:
                        idx2_w = iop.tile([128, cwn * 8], I16, tag="idx2w")
                        nc.sync.dma_start(idx2_w[:],
                                          idx2_d[:, cw0 * 8:(cw0 + cwn) * 8])
                        segv2_w = iop.tile([128, 2, cwn], F32, tag="segv2w")
                        nc.sync.dma_start(segv2_w[:],
                                          segv2_d[:, :, cw0:cw0 + cwn])
                    t_all = iop.tile([1, WIN], F32, tag="tall")
                    nc.sync.dma_start(t_all[:, :span],
                                      t_dram[:, w * WIN:w * WIN + span])
                    if "only_mem" in ablate:
                        agg2_ps = psp2.tile([1, WIN], F32, space="PSUM",
                                            tag="agg")
                        nc.vector.memset(agg2_ps[:, :], 0.0)
                        out_sb = iop.tile([1, WIN], F32, tag="outsb")
                        nc.vector.tensor_tensor(out=out_sb[:, :span],
                                                in0=agg2_ps[:, :span],
                                                in1=t_all[:, :span],
                                                op=mybir.AluOpType.add)
                        nc.sync.dma_start(out_d[:, w * WIN:w * WIN + span],
                                          out_sb[:, :span])
                    else:
                        nc.sync.dma_start(out_d[:, w * WIN:w * WIN + span],
                                          t_all[:, :span])
            for w in range(NWIN if "nol2" not in ablate else 0):
                span = span_of(w)
                cw0 = co2[w][0]
                cwn = co2[w][cfg.NCHUNK2 - 1] + R2[w][cfg.NCHUNK2 - 1] - cw0

                idx2_w = iop.tile([128, cwn * 8], I16, tag="idx2w")
                nc.sync.dma_start(idx2_w[:], idx2_d[:, cw0 * 8:(cw0 + cwn) * 8])
                segv2_w = iop.tile([128, 2, cwn], F32, tag="segv2w")
                nc.sync.dma_start(segv2_w[:], segv2_d[:, :, cw0:cw0 + cwn])
                t_row = iop.tile([1, WIN], F32, tag="trow")
                nc.sync.dma_start(t_row[:, :span],
                                  t_dram[:, w * WIN:w * WIN + span])

                agg2_ps = psp2.tile([1, WIN], F32, space="PSUM", tag="agg")
                nc.vector.memset(agg2_ps[:, :], 0.0)

                mm = []
                for c in range(cfg.NCHUNK2):
                    if "nog2" in ablate:
                        break
                    r = R2[w][c]
                    o = co2[w][c] - cw0
                    wg2 = gatp.tile([128, r, 64], F32, tag=f"wg2{c % 2}")
                    nc.gpsimd.dma_gather(
                        out_ap=wg2[:, :, :],
                        in_ap=s_big[c * cfg.CHUNK2:
                                    min((c + 1) * cfg.CHUNK2, cfg.NROW2), :],
                        idxs_ap=idx2_w[:, o * 8:(o + r) * 8],
                        num_idxs=r * 128, num_idxs_reg=r * 128, elem_size=64,
                        single_packet=False, queue_num=c % cfg.NSWQ)
                    for b0 in range(0, r if "g2only" not in ablate else 0, COLB):
                        nb = min(COLB, r - b0)
                        B = bbp.tile([128, COLB, SEGW], F32, tag="B2")
                        seg_b = segv2_w[:, 0, o + b0:o + b0 + nb].unsqueeze(2) \
                            .to_broadcast([128, nb, SEGW])
                        v_b = segv2_w[:, 1, o + b0:o + b0 + nb].unsqueeze(2) \
                            .to_broadcast([128, nb, SEGW])
                        iotas_b = iota32.unsqueeze(1).to_broadcast([128, nb, SEGW])
                        nc.vector.tensor_tensor(out=B[:, :nb, :], in0=seg_b,
                                                in1=iotas_b,
                                                op=mybir.AluOpType.is_equal)
                        nc.vector.tensor_tensor(out=B[:, :nb, :], in0=B[:, :nb, :],
                                                in1=v_b, op=mybir.AluOpType.mult)
                        for t in range(nb):
                            j = b0 + t
                            f = F2[w][c][j]
                            sw = min(SEGW, span - f)
                            mm.append((wg2, j, B, t, f, sw))
                for q, (wg2, j, B, t, f, sw) in enumerate(mm):
                    nc.tensor.matmul(out=agg2_ps[:, f:f + sw],
                                     lhsT=wg2[:, j, 0:1], rhs=B[:, t, :sw],
                                     start=False, stop=(q == len(mm) - 1),
                                     skip_group_check=True)

                out_sb = iop.tile([1, WIN], F32, tag="outsb")
                nc.vector.tensor_tensor(out=out_sb[:, :span],
                                        in0=agg2_ps[:, :span],
                                        in1=t_row[:, :span],
                                        op=mybir.AluOpType.add)
                nc.sync.dma_start(out_d[:, w * WIN:w * WIN + span],
                                  out_sb[:, :span])

    nc.compile()
    return nc


def kernel(x, edge_index, batch, fc_W, fc_b, Wl1, bl1, Wr1, Wl2, bl2, Wr2,
           head_W, head_b, cfg=None):
    cfg = cfg or Cfg(n_nodes=x.shape[0], in_dim=x.shape[1], hid=fc_W.shape[0])
    x = np.asarray(x, dtype=np.float32)
    src = np.asarray(edge_index[0], dtype=np.int64)
    dst = np.asarray(edge_index[1], dtype=np.int64)
    fc_W, fc_b, Wl1, bl1, Wr1, Wl2, bl2, Wr2, head_W, head_b = [
        np.asarray(a, dtype=np.float32)
        for a in (fc_W, fc_b, Wl1, bl1, Wr1, Wl2, bl2, Wr2, head_W, head_b)]

    t0 = time.time()
    p1, p2 = plan(cfg, src, dst)
    timings["plan_s"] = time.time() - t0
    wt, wcols = pack_weights(cfg, fc_W, fc_b, Wl1, bl1, Wr1,
                             Wl2, bl2, Wr2, head_W, head_b)

    t0 = time.time()
    ncb = build(cfg, p1, p2, wcols, wt.shape[1])
    timings["build_s"] = time.time() - t0

    in_maps = []
    for k in range(cfg.NC):
        in_maps.append({
            "wt": wt, "idx1": p1["idx"][k], "segv1": p1["segv"][k],
            "idx2": p2["idx"][k], "segv2": p2["segv"][k],
            "x_my": x[k * cfg.NPC:(k + 1) * cfg.NPC],
        })
    t0 = time.time()
    res = run_bass_kernel_spmd(ncb, in_maps, core_ids=list(range(cfg.NC)))
    timings["run_s"] = time.time() - t0
    timings["hw_ns"] = res.exec_time_ns
    out = np.concatenate([r["out"][0] for r in res.results], axis=0)
    return out.reshape(cfg.N, 1).astype(np.float32)


# revision 44
# speedup vs baseline: 1.9730x; 1.0014x over previous
"""Trainium2 Bass kernel for CustomGNN (fc+relu -> SAGEConv+relu -> SAGEConv -> head).

Single-NEFF SPMD design (8 NeuronCores, one program, per-core data):
  - nodes sharded in 8 contiguous chunks of 12500 (dst-partitioned edges).
  - fc stage sharded: each core computes h0 for its own nodes, then an
    AllGather builds the full [100000, 64] h0 table in every core's HBM.
  - SAGE layer 1: per 512-dst window, edges (sorted by dst, packed into
    128-slot columns) gather h0[src] rows (256B) with dma_gather from 4 src
    chunks (int16 index range); segment-sum via PE matmuls against one-hot
    inv-deg matrices built on DVE (is_equal vs iota over 32-wide sliding seg
    windows); then zT = Wl1 @ aggT + Wr1 @ h0T + bl1, relu -> h1T.
  - SAGE layer 2 + head collapsed to scalars (linearity of the head):
      out[v] = invdeg_v * sum_{u->v} s[u] + t[v]
      s[u] = h1[u] . (head_W @ Wl2),  t[v] = h1[v] . (head_W @ Wr2) + c0
    s is AllGathered (50KB/rank, padded shard stride 12544), expanded into a
    [100352, 64] row-table (s at column 0 of each 256B row, one strided DMA
    per chunk), then a second window pass mirrors layer 1: dma_gather 256B
    rows from 4 int16 chunks and PE segment-sum with 1-wide lhsT (column 0),
    accumulating straight into a [1, 512] PSUM row added to t.

All index-derived structure (column quotas R, seg-window starts F) is computed
on the host from edge_index and baked into the program; quotas are max-merged
across the 8 cores so the SPMD program is identical on every core.
"""
import math
import time
import numpy as np

from concourse import bacc, mybir
from concourse.tile import TileContext
from concourse.bass_utils import run_bass_kernel_spmd

timings = {}

NSWQ = 2

F32 = mybir.dt.float32
I16 = mybir.dt.int16


class Cfg:
    def __init__(self, n_nodes=100000, in_dim=128, hid=64, ncores=8,
                 win=512, nchunk=4, segw=32, colb=32, xb=4, gb2=48, nswq=4,
                 gbufs=2, iobufs=3):
        assert n_nodes % ncores == 0
        self.N = n_nodes
        self.D = in_dim
        self.H = hid
        self.NC = ncores
        self.NPC = n_nodes // ncores
        self.WIN = win
        self.NWIN = math.ceil(self.NPC / win)
        self.NCHUNK = nchunk
        self.CHUNK = math.ceil(n_nodes / nchunk)
        assert self.CHUNK <= 32767
        self.SEGW = segw
        self.COLB = colb
        self.XB = xb
        self.GB2 = gb2                        # layer-2 gather block (columns)
        self.NSWQ = nswq
        self.GBUFS = gbufs
        self.IOBUFS = iobufs
        self.NBLK = math.ceil(self.NPC / 128)
        self.SPAD = self.NBLK * 128           # padded shard stride (12544)
        assert self.SPAD % 64 == 0
        self.NB2 = self.SPAD * ncores // 64   # s AG output rows ([NB2, 64])
        self.NROW2 = self.SPAD * ncores       # expanded s table rows
        self.NCHUNK2 = nchunk
        self.CHUNK2 = math.ceil(self.NROW2 / self.NCHUNK2)
        assert self.CHUNK2 <= 32767
        # split-s overlap: windows [0, HWIN) AllGather early so layer-2
        # pass A runs concurrently with layer-1 windows [HWIN, NWIN)
        self.HWIN = self.NWIN // 2 + 1
        self.LOCA = min(self.HWIN * win, self.NPC)
        assert self.LOCA % 128 == 0
        self.BLKA = self.LOCA // 128
        self.LOCB = self.NPC - self.LOCA
        self.BLKB = math.ceil(self.LOCB / 128)
        self.SPADB = self.BLKB * 128
        assert self.LOCA % 64 == 0 and self.SPADB % 64 == 0
        self.NROWA = self.LOCA * ncores
        self.NROWB = self.SPADB * ncores
        self.NCHUNKA = math.ceil(self.NROWA / 32767)
        self.CHUNKA = math.ceil(self.NROWA / self.NCHUNKA / 64) * 64
        self.NCHUNKB = math.ceil(self.NROWB / 32767)
        self.CHUNKB = math.ceil(self.NROWB / self.NCHUNKB / 64) * 64


def _schedule_F(R, span, segw):
    if R <= 1:
        return [0] * max(R, 1)
    top = max(span - segw, 0)
    return [min((j * top) // (R - 1), top) for j in range(R)]


def _schedule_F_adaptive(segs_by_core, r, span, segw):
    """Place col windows at worst-case rank positions across cores: col j
    targets rank ~j*128; window start bounded above by every core's seg at
    that rank (no lagging edge) and nudged up to cover the window tail."""
    top = max(span - segw, 0)
    nmax = max((len(s) for s in segs_by_core), default=0)
    if r <= 1 or nmax == 0:
        return [0] * max(r, 1)
    F = []
    prev = 0
    for j in range(r):
        m = (j * (nmax - 1)) // (r - 1)
        hi = top
        lo = 0
        for s in segs_by_core:
            n = len(s)
            if n:
                hi = min(hi, int(s[min(m, n - 1)]))
                lo = max(lo, int(s[min(m + 127, n - 1)]) - (segw - 1))
        f = max(prev, min(hi, top))
        f = max(f, min(lo, top))
        F.append(f)
        prev = f
    return F


def _pack_group(segs, R, F, segw):
    """Greedy-pack sorted segs into R cols of <=128 edges, col j window
    [F[j], F[j]+segw). Returns list of (start, count) per col, or None."""
    n = len(segs)
    ptr = 0
    out = []
    for j in range(R):
        if ptr < n and segs[ptr] < F[j]:
            return None
        hi = np.searchsorted(segs, F[j] + segw, side="left")
        take = min(128, hi - ptr)
        out.append((ptr, take))
        ptr += take
    if ptr != n:
        return None
    return out


def _plan_pass(cfg, dst, seg_all_src, gid_base, nchunk, chunk, tabidx, extra,
               inv=None):
    """Generic planning for one gather+segment-sum pass.

    dst: global dst per edge; chunk: chunk id per edge (grouping + gather
    table split); tabidx: per-edge gather index (relative to chunk base
    handled by caller); extra: dict name -> per-edge f32 array baked beside
    (seg, invdeg) into the segv tensor.
    Returns structure dict + per-core idx (int16, wrapped) and segv arrays.
    """
    N, NC, NPC, WIN, NWIN, SEGW = cfg.N, cfg.NC, cfg.NPC, cfg.WIN, cfg.NWIN, cfg.SEGW

    if inv is None:
        deg = np.bincount(dst, minlength=N)
        inv = (1.0 / np.maximum(deg, 1)).astype(np.float32)

    core = dst // NPC
    local = dst - core * NPC
    win = local // WIN
    seg = (local - win * WIN).astype(np.int32)

    gid = ((core * NWIN + win) * nchunk + chunk).astype(np.int64)
    order = np.lexsort((seg, gid))
    gids = gid[order]
    segs_all = seg[order]
    tab_all = tabidx[order]
    invdst_all = inv[dst[order]].astype(np.float32)
    extra_all = {k: v[order] for k, v in extra.items()}

    ngroups = NC * NWIN * nchunk
    counts = np.bincount(gids, minlength=ngroups)
    starts = np.zeros(ngroups + 1, dtype=np.int64)
    np.cumsum(counts, out=starts[1:])

    def span_of(w):
        return min(WIN, NPC - w * WIN)

    R = [[0] * nchunk for _ in range(NWIN)]
    F = [[None] * nchunk for _ in range(NWIN)]
    packs = {}
    for w in range(NWIN):
        span = span_of(w)
        for c in range(nchunk):
            r = 1
            segs_by_core = []
            for k in range(NC):
                g = (k * NWIN + w) * nchunk + c
                segs_by_core.append(segs_all[starts[g]:starts[g + 1]])
                r = max(r, math.ceil(max(counts[g], 1) / 128))
            while True:
                done = False
                for f in (_schedule_F_adaptive(segs_by_core, r, span, SEGW),
                          _schedule_F(r, span, SEGW)):
                    ok = True
                    for k in range(NC):
                        p = _pack_group(segs_by_core[k], r, f, SEGW)
                        if p is None:
                            ok = False
                            break
                        packs[(k, w, c)] = p
                    if ok:
                        done = True
                        break
                if done:
                    break
                r += 1
            R[w][c] = r
            F[w][c] = f

    col_off = [[0] * nchunk for _ in range(NWIN)]
    tot = 0
    for w in range(NWIN):
        for c in range(nchunk):
            col_off[w][c] = tot
            tot += R[w][c]
    totcols = tot

    nex = 2 + len(extra)
    exnames = sorted(extra.keys())
    idx_wrapped = []
    segv = []
    for k in range(NC):
        idxk = np.zeros((totcols, 128), dtype=np.int16)
        vals = np.zeros((nex, totcols, 128), dtype=np.float32)
        for w in range(NWIN):
            for c in range(nchunk):
                g = (k * NWIN + w) * nchunk + c
                s0 = starts[g]
                f = F[w][c]
                for j, (ptr, take) in enumerate(packs[(k, w, c)]):
                    if take <= 0:
                        continue
                    col = col_off[w][c] + j
                    sl = slice(s0 + ptr, s0 + ptr + take)
                    idxk[col, :take] = tab_all[sl].astype(np.int16)
                    vals[0, col, :take] = (segs_all[sl] - f[j]).astype(np.float32)
                    vals[1, col, :take] = invdst_all[sl]
                    for ei, en in enumerate(exnames):
                        vals[2 + ei, col, :take] = extra_all[en][sl]
        iw = idxk.reshape(totcols, 8, 16).transpose(2, 0, 1).reshape(16, totcols * 8)
        idx_wrapped.append(np.tile(iw, (8, 1)))
        segv.append(np.ascontiguousarray(vals.transpose(2, 0, 1)))  # [128, nex, totcols]

    return dict(R=R, F=F, col_off=col_off, totcols=totcols, nex=nex,
                idx=idx_wrapped, segv=segv, span_of=span_of)


def plan(cfg, src, dst):
    """Host planning for both gather passes."""
    chunk1 = src // cfg.CHUNK
    tab1 = (src - chunk1 * cfg.CHUNK).astype(np.int64)
    p1 = _plan_pass(cfg, dst, None, 0, cfg.NCHUNK, chunk1, tab1, {})

    # layer 2, split by src shard-local half so pass A overlaps layer 1:
    inv = (1.0 / np.maximum(np.bincount(dst, minlength=cfg.N), 1)) \
        .astype(np.float32)
    local = src % cfg.NPC
    kc = src // cfg.NPC
    maskA = local < cfg.LOCA
    sa, da = src[maskA], dst[maskA]
    rowA = (sa // cfg.NPC) * cfg.LOCA + (sa % cfg.NPC)
    cA = rowA // cfg.CHUNKA
    tA = (rowA - cA * cfg.CHUNKA).astype(np.int64)
    p2a = _plan_pass(cfg, da, None, 0, cfg.NCHUNKA, cA, tA, {}, inv=inv)
    sb, db = src[~maskA], dst[~maskA]
    rowB = (sb // cfg.NPC) * cfg.SPADB + (sb % cfg.NPC - cfg.LOCA)
    cB = rowB // cfg.CHUNKB
    tB = (rowB - cB * cfg.CHUNKB).astype(np.int64)
    p2b = _plan_pass(cfg, db, None, 0, cfg.NCHUNKB, cB, tB, {}, inv=inv)
    return p1, p2a, p2b


def pack_weights(cfg, fc_W, fc_b, Wl1, bl1, Wr1, Wl2, bl2, Wr2, head_W, head_b):
    """One [128, WCOLS] f32 tensor with all constants."""
    D, H, SEGW = cfg.D, cfg.H, cfg.SEGW
    alpha = (head_W @ Wl2)[0]            # [H]
    beta = (head_W @ Wr2)[0]             # [H]
    c0 = float(head_b[0] + head_W[0] @ bl2)

    cols = {}
    seg = []

    def add(name, arr):
        arr = np.asarray(arr, np.float32)
        assert arr.ndim == 2 and arr.shape[0] <= 128
        cols[name] = sum(a.shape[1] for a in seg)
        pad = np.zeros((128, arr.shape[1]), np.float32)
        pad[:arr.shape[0]] = arr
        seg.append(pad)

    add("ident", np.eye(D, dtype=np.float32))
    add("iota32", np.tile(np.arange(SEGW, dtype=np.float32), (128, 1)))
    add("iota64", np.tile(np.arange(64, dtype=np.float32), (128, 1)))
    add("fcWT", fc_W.T)                  # [D, H]
    add("Wl1T", Wl1.T)                   # [H, H]
    add("Wr1T", Wr1.T)
    add("alpha", alpha[:, None])         # [H, 1]
    add("beta", beta[:, None])
    add("bl1", bl1[:, None])
    add("fcb", fc_b[:, None])
    add("ones", np.ones((H, 1), np.float32))
    add("c0", np.full((1, 1), c0, np.float32))
    wt = np.concatenate(seg, axis=1)
    return wt, cols


def build(cfg, p1, p2a, p2b, wcols, wcols_n, ablate=()):
    N, D, H, NPC, WIN, NWIN = cfg.N, cfg.D, cfg.H, cfg.NPC, cfg.WIN, cfg.NWIN
    NCHUNK, CHUNK, SEGW, COLB, XB = cfg.NCHUNK, cfg.CHUNK, cfg.SEGW, cfg.COLB, cfg.XB
    NBLK, SPAD, NB2, GB2 = cfg.NBLK, cfg.SPAD, cfg.NB2, cfg.GB2
    R1, F1, co1, tc1 = p1["R"], p1["F"], p1["col_off"], p1["totcols"]
    tc2a, tc2b = p2a["totcols"], p2b["totcols"]
    span_of = p1["span_of"]

    nc = bacc.Bacc(None, target_bir_lowering=False, num_swdge_queues=cfg.NSWQ,
                   num_devices=cfg.NC)
    wt_d = nc.dram_tensor("wt", [128, wcols_n], F32, kind="ExternalInput")
    x_d = nc.dram_tensor("x_my", [NPC, D], F32, kind="ExternalInput")
    idx1_d = nc.dram_tensor("idx1", [128, tc1 * 8], I16, kind="ExternalInput")
    segv1_d = nc.dram_tensor("segv1", [128, 2, tc1], F32, kind="ExternalInput")
    idx2a_d = nc.dram_tensor("idx2a", [128, tc2a * 8], I16, kind="ExternalInput")
    segv2a_d = nc.dram_tensor("segv2a", [128, 2, tc2a], F32,
                              kind="ExternalInput")
    idx2b_d = nc.dram_tensor("idx2b", [128, tc2b * 8], I16, kind="ExternalInput")
    segv2b_d = nc.dram_tensor("segv2b", [128, 2, tc2b], F32,
                              kind="ExternalInput")
    out_d = nc.dram_tensor("out", [1, NPC], F32, kind="ExternalOutput")

    rg = [list(range(cfg.NC))]

    with TileContext(nc) as tc:
        with (
            tc.tile_pool(name="dram", bufs=1, space="DRAM") as dramp,
            tc.tile_pool(name="const", bufs=1) as constp,
            tc.tile_pool(name="resid", bufs=1) as resid,
            tc.tile_pool(name="io", bufs=cfg.IOBUFS) as iop,
            tc.tile_pool(name="gat", bufs=cfg.GBUFS) as gatp,
            tc.tile_pool(name="bb", bufs=3) as bbp,
            tc.tile_pool(name="ps", bufs=2, space="PSUM") as psp,
            tc.tile_pool(name="ps2", bufs=2, space="PSUM") as psp2,
        ):
            shared = "Local" if "nocc" in ablate else "Shared"
            ag_in = dramp.tile([NPC, H], F32)
            h0_full = dramp.tile([N, H], F32, addr_space=shared)
            hT_dram = dramp.tile([H, NPC], F32)
            t_dram = dramp.tile([1, NPC], F32)
            outp_dram = dramp.tile([1, NPC], F32)
            s_ag_inA = dramp.tile([cfg.LOCA, 1], F32)
            s_fullA = dramp.tile([cfg.NROWA // 64, 64], F32, addr_space=shared)
            s_bigA = dramp.tile([cfg.NROWA, 64], F32)
            s_ag_inB = dramp.tile([cfg.SPADB, 1], F32)
            s_fullB = dramp.tile([cfg.NROWB // 64, 64], F32, addr_space=shared)
            s_bigB = dramp.tile([cfg.NROWB, 64], F32)

            wt = constp.tile([128, wcols_n], F32)
            nc.sync.dma_start(wt[:], wt_d[:, :])
            ident = wt[:, wcols["ident"]:wcols["ident"] + D]
            iota32 = wt[:, wcols["iota32"]:wcols["iota32"] + SEGW]
            iota64 = wt[:, wcols["iota64"]:wcols["iota64"] + 64]
            fcWT = wt[:D, wcols["fcWT"]:wcols["fcWT"] + H]
            Wl1T = wt[:H, wcols["Wl1T"]:wcols["Wl1T"] + H]
            Wr1T = wt[:H, wcols["Wr1T"]:wcols["Wr1T"] + H]
            alpha = wt[:H, wcols["alpha"]:wcols["alpha"] + 1]
            beta = wt[:H, wcols["beta"]:wcols["beta"] + 1]
            bl1 = wt[:H, wcols["bl1"]:wcols["bl1"] + 1]
            fcb = wt[:H, wcols["fcb"]:wcols["fcb"] + 1]
            ones = wt[:H, wcols["ones"]:wcols["ones"] + 1]
            c0 = wt[0:1, wcols["c0"]:wcols["c0"] + 1]

            # s values, block-column layout: [p, b] = s[node b*128+p];
            # split so the A half's AllGather can fire mid layer-1
            s_sbA = resid.tile([128, cfg.BLKA], F32)
            s_sbB = resid.tile([128, cfg.BLKB], F32)
            nc.vector.memset(s_sbA[:, :], 0.0)
            nc.vector.memset(s_sbB[:, :], 0.0)

            # ---- fc stage over my shard: h0 rows -> ag_in, h0T -> hT_dram --
            nt = math.ceil(NPC / 128)
            for i0 in range(0, nt, XB):
                nb = min(XB, nt - i0)
                rows0 = i0 * 128
                rowsn = min(NPC - rows0, nb * 128)
                xb_t = iop.tile([128, XB, D], F32, tag="xb")
                nfull = rowsn // 128
                if nfull:
                    nc.sync.dma_start(
                        xb_t[:, :nfull, :],
                        x_d[rows0:rows0 + nfull * 128, :].rearrange(
                            "(t p) d -> p t d", p=128))
                rem = rowsn - nfull * 128
                if rem:
                    nc.sync.dma_start(xb_t[:rem, nfull, :],
                                      x_d[rows0 + nfull * 128:rows0 + rowsn, :])
                hb_t = iop.tile([128, XB, H], F32, tag="hb")
                hTb_t = iop.tile([H, XB, 128], F32, tag="hTb")
                for t in range(nb):
                    rows = min(128, NPC - rows0 - t * 128)
                    xT_ps = psp.tile([128, D], F32, space="PSUM", tag="s")
                    nc.tensor.transpose(xT_ps[:, :rows], xb_t[:rows, t, :],
                                        ident[:rows, :rows])
                    xT = iop.tile([128, D], F32, tag="xT_sb")
                    nc.vector.tensor_copy(xT[:], xT_ps[:])
                    hT_ps = psp.tile([H, 128], F32, space="PSUM", tag="s")
                    nc.tensor.matmul(out=hT_ps[:, :rows], lhsT=fcWT, rhs=xT[:, :rows],
                                     start=True, stop=True)
                    nc.scalar.activation(hTb_t[:, t, :rows], hT_ps[:, :rows],
                                         mybir.ActivationFunctionType.Relu, bias=fcb)
                    h_ps = psp.tile([128, H], F32, space="PSUM", tag="s")
                    nc.tensor.transpose(h_ps[:rows, :], hTb_t[:, t, :rows],
                                        ident[:H, :H])
                    nc.vector.tensor_copy(hb_t[:rows, t, :], h_ps[:rows, :])
                nc.sync.dma_start(hT_dram[:, rows0:rows0 + rowsn],
                                  hTb_t[:, :nb, :].rearrange(
                                      "h t p -> h (t p)")[:, :rowsn])
                if nfull:
                    nc.sync.dma_start(
                        ag_in[rows0:rows0 + nfull * 128, :].rearrange(
                            "(t p) d -> p t d", p=128),
                        hb_t[:, :nfull, :])
                if rem:
                    nc.sync.dma_start(ag_in[rows0 + nfull * 128:rows0 + rowsn, :],
                                      hb_t[:rem, nfull, :])

            if "nocc" in ablate:
                for kk in range(cfg.NC):
                    nc.sync.dma_start(h0_full[kk * NPC:(kk + 1) * NPC, :],
                                      ag_in[:, :])
            else:
                nc.gpsimd.collective_compute(
                    "AllGather", mybir.AluOpType.bypass, replica_groups=rg,
                    ins=[ag_in.opt()], outs=[h0_full.opt()])

            # ---- layer-1 window loop -----------------------------------
            for w in range(NWIN):
                span = span_of(w)
                cw0 = co1[w][0]
                cwn = co1[w][NCHUNK - 1] + R1[w][NCHUNK - 1] - cw0

                idx_w = iop.tile([128, cwn * 8], I16, tag="idxw")
                nc.sync.dma_start(idx_w[:], idx1_d[:, cw0 * 8:(cw0 + cwn) * 8])
                segv_w = iop.tile([128, 2, cwn], F32, tag="segvw")
                nc.sync.dma_start(segv_w[:], segv1_d[:, :, cw0:cw0 + cwn])
                hTw = iop.tile([H, WIN], F32, tag="hTw")
                nc.sync.dma_start(hTw[:, :span],
                                  hT_dram[:, w * WIN:w * WIN + span])

                agg_ps = psp2.tile([H, WIN], F32, space="PSUM", tag="agg")
                nc.vector.memset(agg_ps[:, :], 0.0)

                mm = []
                for c in range(NCHUNK):
                    if "nog1" in ablate:
                        break
                    r = R1[w][c]
                    o = co1[w][c] - cw0
                    wg = gatp.tile([128, r, H], F32, tag=f"wg{c % 2}")
                    nc.gpsimd.dma_gather(
                        out_ap=wg[:, :, :],
                        in_ap=h0_full[c * CHUNK:min((c + 1) * CHUNK, N), :],
                        idxs_ap=idx_w[:, o * 8:(o + r) * 8],
                        num_idxs=r * 128, num_idxs_reg=r * 128, elem_size=H,
                        single_packet=False, queue_num=c % cfg.NSWQ)
                    for b0 in range(0, r, COLB):
                        nb = min(COLB, r - b0)
                        B = bbp.tile([128, COLB, SEGW], F32, tag="B")
                        seg_b = segv_w[:, 0, o + b0:o + b0 + nb].unsqueeze(2) \
                            .to_broadcast([128, nb, SEGW])
                        v_b = segv_w[:, 1, o + b0:o + b0 + nb].unsqueeze(2) \
                            .to_broadcast([128, nb, SEGW])
                        iota_b = iota32.unsqueeze(1).to_broadcast([128, nb, SEGW])
                        nc.vector.tensor_tensor(out=B[:, :nb, :], in0=seg_b, in1=iota_b,
                                                op=mybir.AluOpType.is_equal)
                        nc.vector.tensor_tensor(out=B[:, :nb, :], in0=B[:, :nb, :],
                                                in1=v_b, op=mybir.AluOpType.mult)
                        for t in range(nb):
                            j = b0 + t
                            f = F1[w][c][j]
                            sw = min(SEGW, span - f)
                            mm.append((wg, j, B, t, f, sw))
                for q, (wg, j, B, t, f, sw) in enumerate(mm):
                    nc.tensor.matmul(out=agg_ps[:, f:f + sw], lhsT=wg[:, j, :],
                                     rhs=B[:, t, :sw], start=False,
                                     stop=(q == len(mm) - 1), skip_group_check=True)

                aggT = iop.tile([H, WIN], F32, tag="aggT")
                nc.scalar.copy(aggT[:, :span], agg_ps[:, :span])

                z_ps = psp2.tile([H, WIN], F32, space="PSUM", tag="z")
                nc.tensor.matmul(out=z_ps[:, :span], lhsT=Wl1T, rhs=aggT[:, :span],
                                 start=True, stop=False)
                nc.tensor.matmul(out=z_ps[:, :span], lhsT=Wr1T,
                                 rhs=hTw[:, :span], start=False, stop=True)

                h1T_sb = iop.tile([H, WIN], F32, tag="h1T")
                nc.scalar.activation(h1T_sb[:, :span], z_ps[:, :span],
                                     mybir.ActivationFunctionType.Relu, bias=bl1)

                # s per 128-node block (column layout), t as row
                nblk = math.ceil(span / 128)
                s_ps = psp.tile([128, 4], F32, space="PSUM", tag="s")
                for b in range(nblk):
                    rows = min(128, span - b * 128)
                    nc.tensor.matmul(out=s_ps[:rows, b:b + 1],
                                     lhsT=h1T_sb[:, b * 128:b * 128 + rows],
                                     rhs=alpha, start=True, stop=True)
                s_tgt, s_c0 = ((s_sbA, w * 4) if w < cfg.HWIN
                               else (s_sbB, w * 4 - cfg.BLKA))
                if span == WIN:
                    nc.scalar.copy(s_tgt[:, s_c0:s_c0 + nblk], s_ps[:, :nblk])
                else:
                    for b in range(nblk):
                        rows = min(128, span - b * 128)
                        nc.scalar.copy(s_tgt[:rows, s_c0 + b:s_c0 + b + 1],
                                       s_ps[:rows, b:b + 1])
                t_ps = psp.tile([1, WIN], F32, space="PSUM", tag="s")
                nc.tensor.matmul(out=t_ps[:, :span], lhsT=beta,
                                 rhs=h1T_sb[:, :span], start=True, stop=True)
                t_sb = iop.tile([1, WIN], F32, tag="tsb")
                nc.scalar.activation(t_sb[:, :span], t_ps[:, :span],
                                     mybir.ActivationFunctionType.Identity,
                                     bias=c0)
                nc.sync.dma_start(t_dram[:, w * WIN:w * WIN + span],
                                  t_sb[:, :span])

            # ---- split s AllGathers + row-table expansion ---------------
            # (AG-A depends only on layer-1 windows [0, HWIN), so pass A's
            # gathers overlap layer-1 windows [HWIN, NWIN))
            for s_sb_h, ag_in_h, full_h, big_h, nch, chk in (
                (s_sbA, s_ag_inA, s_fullA, s_bigA, cfg.NCHUNKA, cfg.CHUNKA),
                (s_sbB, s_ag_inB, s_fullB, s_bigB, cfg.NCHUNKB, cfg.CHUNKB),
            ):
                nc.sync.dma_start(
                    ag_in_h[:, :].rearrange("(t p) o -> p (t o)", p=128),
                    s_sb_h[:, :])
                if "nocc" in ablate:
                    nr = ag_in_h.shape[0] // 64
                    for kk in range(cfg.NC):
                        nc.sync.dma_start(
                            full_h[kk * nr:(kk + 1) * nr, :],
                            ag_in_h[:, :].rearrange("(r c) o -> r (c o)", c=64))
                else:
                    nc.gpsimd.collective_compute(
                        "AllGather", mybir.AluOpType.bypass, replica_groups=rg,
                        ins=[ag_in_h.opt()], outs=[full_h.opt()])
                # expand: big[u, 0] = s_linear[u]; chunked strided DMAs keep
                # each transfer's dims under the 16-bit ISA field
                nrc = chk // 64
                for c in range(nch):
                    nc.sync.dma_start(
                        big_h[c * chk:(c + 1) * chk, 0:1],
                        full_h[c * nrc:(c + 1) * nrc, :]
                        .rearrange("r c -> (r c)").unsqueeze(1))

            # ---- layer-2 window loops (scalar messages), two src-half
            # passes: A (overlaps layer-1 tail) writes partial sums to
            # outp_dram; B adds them and emits the output ----------------
            if "nol2" in ablate:
                for w in range(NWIN):
                    span = span_of(w)
                    t_all = iop.tile([1, WIN], F32, tag="tall")
                    nc.sync.dma_start(t_all[:, :span],
                                      t_dram[:, w * WIN:w * WIN + span])
                    nc.sync.dma_start(out_d[:, w * WIN:w * WIN + span],
                                      t_all[:, :span])

            parts = () if "nol2" in ablate else (
                (p2a, idx2a_d, segv2a_d, s_bigA, cfg.NCHUNKA, cfg.CHUNKA,
                 cfg.NROWA, t_dram, outp_dram),
                (p2b, idx2b_d, segv2b_d, s_bigB, cfg.NCHUNKB, cfg.CHUNKB,
                 cfg.NROWB, outp_dram, out_d),
            )
            for p2, idx2_d, segv2_d, big_h, nch, chk, nrow, base_d, dst_d \
                    in parts:
                R2, F2, co2 = p2["R"], p2["F"], p2["col_off"]
                for w in range(NWIN):
                    span = span_of(w)
                    cw0 = co2[w][0]
                    cwn = co2[w][nch - 1] + R2[w][nch - 1] - cw0

                    idx2_w = iop.tile([128, cwn * 8], I16, tag="idx2w")
                    nc.sync.dma_start(idx2_w[:],
                                      idx2_d[:, cw0 * 8:(cw0 + cwn) * 8])
                    segv2_w = iop.tile([128, 2, cwn], F32, tag="segv2w")
                    nc.sync.dma_start(segv2_w[:], segv2_d[:, :, cw0:cw0 + cwn])
                    base_row = iop.tile([1, WIN], F32, tag="trow")
                    nc.sync.dma_start(base_row[:, :span],
                                      base_d[:, w * WIN:w * WIN + span])

                    agg2_ps = psp2.tile([1, WIN], F32, space="PSUM", tag="agg2")
                    nc.vector.memset(agg2_ps[:, :], 0.0)

                    mm = []
                    for c in range(nch):
                        r = R2[w][c]
                        o = co2[w][c] - cw0
                        wg2 = gatp.tile([128, r, 64], F32, tag=f"wg2{c % 2}")
                        nc.gpsimd.dma_gather(
                            out_ap=wg2[:, :, :],
                            in_ap=big_h[c * chk:min((c + 1) * chk, nrow), :],
                            idxs_ap=idx2_w[:, o * 8:(o + r) * 8],
                            num_idxs=r * 128, num_idxs_reg=r * 128,
                            elem_size=64, single_packet=False,
                            queue_num=c % cfg.NSWQ)
                        for b0 in range(0, r, COLB):
                            nb = min(COLB, r - b0)
                            B = bbp.tile([128, COLB, SEGW], F32, tag="B2")
                            seg_b = segv2_w[:, 0, o + b0:o + b0 + nb] \
                                .unsqueeze(2).to_broadcast([128, nb, SEGW])
                            v_b = segv2_w[:, 1, o + b0:o + b0 + nb] \
                                .unsqueeze(2).to_broadcast([128, nb, SEGW])
                            iotas_b = iota32.unsqueeze(1) \
                                .to_broadcast([128, nb, SEGW])
                            nc.vector.tensor_tensor(out=B[:, :nb, :], in0=seg_b,
                                                    in1=iotas_b,
                                                    op=mybir.AluOpType.is_equal)
                            nc.vector.tensor_tensor(out=B[:, :nb, :],
                                                    in0=B[:, :nb, :], in1=v_b,
                                                    op=mybir.AluOpType.mult)
                            for t in range(nb):
                                j = b0 + t
                                f = F2[w][c][j]
                                sw = min(SEGW, span - f)
                                mm.append((wg2, j, B, t, f, sw))
                    for q, (wg2, j, B, t, f, sw) in enumerate(mm):
                        nc.tensor.matmul(out=agg2_ps[:, f:f + sw],
                                         lhsT=wg2[:, j, 0:1], rhs=B[:, t, :sw],
                                         start=False, stop=(q == len(mm) - 1),
                                         skip_group_check=True)

                    out_sb = iop.tile([1, WIN], F32, tag="outsb")
                    nc.vector.tensor_tensor(out=out_sb[:, :span],
                                            in0=agg2_ps[:, :span],
                                            in1=base_row[:, :span],
                                            op=mybir.AluOpType.add)
                    nc.sync.dma_start(dst_d[:, w * WIN:w * WIN + span],
                                      out_sb[:, :span])

    nc.compile()
    return nc


def kernel(x, edge_index, batch, fc_W, fc_b, Wl1, bl1, Wr1, Wl2, bl2, Wr2,
           head_W, head_b, cfg=None):
    cfg = cfg or Cfg(n_nodes=x.shape[0], in_dim=x.shape[1], hid=fc_W.shape[0])
    x = np.asarray(x, dtype=np.float32)
    src = np.asarray(edge_index[0], dtype=np.int64)
    dst = np.asarray(edge_index[1], dtype=np.int64)
    fc_W, fc_b, Wl1, bl1, Wr1, Wl2, bl2, Wr2, head_W, head_b = [
        np.asarray(a, dtype=np.float32)
        for a in (fc_W, fc_b, Wl1, bl1, Wr1, Wl2, bl2, Wr2, head_W, head_b)]

    t0 = time.time()
    p1, p2a, p2b = plan(cfg, src, dst)
    timings["plan_s"] = time.time() - t0
    wt, wcols = pack_weights(cfg, fc_W, fc_b, Wl1, bl1, Wr1,
                             Wl2, bl2, Wr2, head_W, head_b)

    t0 = time.time()
    ncb = build(cfg, p1, p2a, p2b, wcols, wt.shape[1])
    timings["build_s"] = time.time() - t0

    in_maps = []
    for k in range(cfg.NC):
        in_maps.append({
            "wt": wt, "idx1": p1["idx"][k], "segv1": p1["segv"][k],
            "idx2a": p2a["idx"][k], "segv2a": p2a["segv"][k],
            "idx2b": p2b["idx"][k], "segv2b": p2b["segv"][k],
            "x_my": x[k * cfg.NPC:(k + 1) * cfg.NPC],
        })
    t0 = time.time()
    res = run_bass_kernel_spmd(ncb, in_maps, core_ids=list(range(cfg.NC)))
    timings["run_s"] = time.time() - t0
    timings["hw_ns"] = res.exec_time_ns
    out = np.concatenate([r["out"][0] for r in res.results], axis=0)
    return out.reshape(cfg.N, 1).astype(np.float32)
